# revision 1
# baseline (speedup 1.0000x reference)
"""Multi-head attention (B=2, S=2048, D=768, H=12) on 8 trn2 NeuronCores.

Sharding: batch x head-group data/tensor parallel. Core c = b*4+g handles
batch b and heads [3g, 3g+3) (a 192-wide slice of the QKV projections and
the matching 192-row slice of Wo). Each core emits a partial [2048, 768]
output; the host sums the 4 head-group partials per batch and adds bo.

Device layout notes:
- Inputs are transposed on host to [d_model, seq] and cast to fp16 so the
  TensorEngine (which contracts over the partition dim) can consume them
  directly; all matmuls run on fp16 operands with fp32 PSUM accumulation.
- Attention works on transposed scores sT[k, q] so softmax's sum over k
  becomes a matmul reduction: v is augmented with a ones column, so the
  ctx matmul yields both ctx^T and the softmax denominator in one pass.
  exp() needs no max-subtraction: |scores/8| <= ~11 for this problem.
- Normalization multiplies ctx^T by 1/denom broadcast across partitions
  (GPSIMD partition_broadcast), then the output projection runs from
  ctx^T directly.
- Heads 0/1 live at SBUF partitions 0-63/64-127 so their score matmuls
  land in different PE row groups and overlap; head 2's operands are
  mirrored into both halves for the same reason.
- The output projection for q-tile j is emitted after q-tile j+1's
  attention so the PE stream never stalls on the normalize chain.
"""

import numpy as np

D_MODEL = 768
NUM_HEADS = 12
D_K = 64
B = 2
S = 2048
N_CORES = 8
G = 4              # head groups (cores per batch)
GW = D_MODEL // G  # 192 features per group = 3 heads
HPG = 3            # heads per group
DC = D_MODEL // 128  # 6 d_model chunks
QT = 512           # q-tile width
NQT = S // QT      # 4
KC = S // 128      # 16 k chunks
ST = S // 128      # 16 seq tiles
WPK = 3 * DC * GW + 2 * D_MODEL  # packed weights columns: 4992
BPK = 8            # packed bias columns

_PROGRAM = None


def _build_program():
    from concourse import bacc, tile
    import concourse.mybir as mybir

    f16 = mybir.dt.float16
    f32 = mybir.dt.float32
    Exp = mybir.ActivationFunctionType.Exp
    mult = mybir.AluOpType.mult

    nc = bacc.Bacc("TRN2", target_bir_lowering=False, debug=False,
                   enable_asserts=False)

    xqT = nc.dram_tensor("xqT", [D_MODEL, S], f16, kind="ExternalInput")
    xkT = nc.dram_tensor("xkT", [D_MODEL, S], f16, kind="ExternalInput")
    xvT = nc.dram_tensor("xvT", [D_MODEL, S], f16, kind="ExternalInput")
    wpk = nc.dram_tensor("wpk", [128, WPK], f16, kind="ExternalInput")
    bpk = nc.dram_tensor("bpk", [128, BPK], f32, kind="ExternalInput")
    out = nc.dram_tensor("out", [S, D_MODEL], f32, kind="ExternalOutput")

    with tile.TileContext(nc) as tc:
        with tc.tile_pool(name="const", bufs=1) as cp, \
             tc.tile_pool(name="expp", bufs=6) as ep, \
             tc.tile_pool(name="normp", bufs=2) as np_, \
             tc.tile_pool(name="outp", bufs=2) as op, \
             tc.tile_pool(name="ps_s", bufs=2, space="PSUM") as ps_s, \
             tc.tile_pool(name="ps_c", bufs=3, space="PSUM") as ps_c, \
             tc.tile_pool(name="ps_o", bufs=1, space="PSUM") as ps_o:

            # ---- packed weights + biases. wk occupies the first
            # columns and ships in its own DMA so the k-projection can
            # start as soon as possible ----
            wps = cp.tile([128, WPK], f16, name="wps")
            nc.sync.dma_start(out=wps[:, 0:DC * GW], in_=wpk[:, 0:DC * GW])
            bps = cp.tile([128, BPK], f32, name="bps")
            nc.sync.dma_start(out=bps[:], in_=bpk[:])
            wk_sb = [wps[:, d * GW:(d + 1) * GW] for d in range(DC)]
            wq_sb = [wps[:, DC * GW + d * GW:DC * GW + (d + 1) * GW]
                     for d in range(DC)]
            wv_sb = [wps[:, 2 * DC * GW + d * GW:2 * DC * GW + (d + 1) * GW]
                     for d in range(DC)]
            wo_a = wps[:, 3 * DC * GW:3 * DC * GW + D_MODEL]
            wo_b = wps[0:64, 3 * DC * GW + D_MODEL:WPK]
            bq_a, bq_b = bps[:, 0:1], bps[0:64, 1:2]
            bk_a, bk_b = bps[:, 2:3], bps[0:64, 3:4]
            bv_h = [bps[0:64, 4 + h:5 + h] for h in range(HPG)]

            # ---- inputs: k first, then v, then q (attention needs full
            # kT and v before it can start, q only per-tile) ----
            xq_sb, xk_sb, xv_sb = [], [], []
            for d in range(DC):
                t = cp.tile([128, S], f16, name=f"xk{d}")
                nc.sync.dma_start(out=t[:], in_=xkT[d * 128:(d + 1) * 128, :])
                xk_sb.append(t)
            # remaining weights (wq/wv/wo) ship after xk so the k
            # projection's data isn't stuck behind them in the DMA queue
            nc.sync.dma_start(out=wps[:, DC * GW:WPK],
                              in_=wpk[:, DC * GW:WPK])
            for d in range(DC):
                t = cp.tile([128, S], f16, name=f"xq{d}")
                nc.sync.dma_start(out=t[:], in_=xqT[d * 128:(d + 1) * 128, :])
                xq_sb.append(t)
            for d in range(DC):
                t = cp.tile([128, S], f16, name=f"xv{d}")
                nc.sync.dma_start(out=t[:], in_=xvT[d * 128:(d + 1) * 128, :])
                xv_sb.append(t)

            # ---- projections. Order: kT, v, qT (dependency order) ----
            qT_a = cp.tile([128, S], f16, name="qT_a")
            qT_b = cp.tile([128, S], f16, name="qT_b")
            kT_a = cp.tile([128, S], f16, name="kT_a")
            kT_b = cp.tile([128, S], f16, name="kT_b")

            def proj_passA(x_sb, w_sb, b_a, dst_a):
                # features 0:128 (heads 0+1), d-outer accumulation: each
                # input chunk is consumed as it arrives from HBM. The
                # attention S-pool is idle here, so borrow its slots.
                pj = [ps_s.tile([128, 2 * QT], f32, name="S", tag="s")
                      for _ in range(2)]
                for d in range(DC):
                    for j2 in range(2):
                        for n in range(2):
                            cs = slice(j2 * 1024 + n * QT,
                                       j2 * 1024 + (n + 1) * QT)
                            nc.tensor.matmul(
                                pj[j2][:, n * QT:(n + 1) * QT],
                                lhsT=w_sb[d][:, 0:128], rhs=x_sb[d][:, cs],
                                start=(d == 0), stop=(d == DC - 1))
                for j2 in range(2):
                    js = slice(j2 * 1024, (j2 + 1) * 1024)
                    nc.vector.tensor_scalar_add(dst_a[:, js], pj[j2][:], b_a)

            def proj_passB(x_sb, w_sb, b_b, dst_b):
                # features 128:192 (head 2): emitted after attention has
                # started, so use the spare ps_c slot in 512-wide chunks
                for n4 in range(4):
                    cs = slice(n4 * QT, (n4 + 1) * QT)
                    pj = ps_c.tile([64, QT], f32, name="pj", tag="c")
                    for d in range(DC):
                        nc.tensor.matmul(pj[:], lhsT=w_sb[d][:, 128:GW],
                                         rhs=x_sb[d][:, cs],
                                         start=(d == 0), stop=(d == DC - 1))
                    nc.vector.tensor_scalar_add(dst_b[0:64, cs], pj[:], b_b)
                # mirror the 64-row b-half into partitions 64-127 so head-2
                # score matmuls can alternate PE row groups (pairing)
                nc.sync.dma_start(out=dst_b[64:128, :], in_=dst_b[0:64, :])

            proj_passA(xk_sb, wk_sb, bk_a, kT_a)
            proj_passA(xq_sb, wq_sb, bq_a, qT_a)

            # v projection (natural layout) + ones column per head.
            # Emitted per seq-tile, fused into q-tile 0's attention loop so
            # the ACT exp stream starts before v finishes projecting.
            v_sb = [None] * ST

            def v_proj(st):
                rs = slice(st * 128, (st + 1) * 128)
                pv = ps_c.tile([128, GW], f32, name="pj", tag="c")
                for d in range(DC):
                    nc.tensor.matmul(pv[:], lhsT=xv_sb[d][:, rs],
                                     rhs=wv_sb[d][:],
                                     start=(d == 0), stop=(d == DC - 1))
                vt = cp.tile([128, HPG, D_K + 1], f16, name=f"vsb{st}")
                nc.vector.tensor_copy(out=vt[:, :, 0:D_K],
                                      in_=pv.rearrange("p (h w) -> p h w",
                                                       h=HPG))
                nc.vector.memset(vt[:, :, D_K:D_K + 1], 1.0)
                v_sb[st] = vt

            # ---- attention (transposed scores) + output projection ----
            # per-q-tile ctx tiles: a single [*, S] tile would make the
            # output projection of q-tile j falsely depend on q-tile j+1's
            # normalize writes (coarse tile deps)
            ctxT_a = [cp.tile([128, QT], f16, name=f"ctxTa{j}")
                      for j in range(NQT)]
            ctxT_b = [cp.tile([64, QT], f16, name=f"ctxTb{j}")
                      for j in range(NQT)]

            def head_slices(h, qt):
                if h == 0:
                    return kT_a[0:64], qT_a[0:64], ctxT_a[qt][0:64]
                if h == 1:
                    return kT_a[64:128], qT_a[64:128], ctxT_a[qt][64:128]
                return kT_b[0:64], qT_b[0:64], ctxT_b[qt][0:64]

            def normalize(C, h, qt):
                # ctxT = C[0:64] * (1/denom) + bv.  reciprocal_approx_fast
                # must read SBUF (garbage from PSUM on HW), so stage the
                # denominator row through SBUF first.
                _, _, ctx_dst = head_slices(h, qt)
                den = np_.tile([1, QT], f32, name="den")
                nc.vector.tensor_copy(out=den[:], in_=C[D_K:D_K + 1, :])
                r = np_.tile([1, QT], f32, name="r")
                nc.vector.reciprocal_approx_fast(out=r[:], in_=den[:])
                bc = np_.tile([128, QT], f32, name="bc")
                nc.gpsimd.partition_broadcast(bc[:], r[:])
                base = 64 if h == 1 else 0
                nc.vector.tensor_tensor(out=ctx_dst[:],
                                        in0=C[0:D_K, :],
                                        in1=bc[base:base + D_K, :],
                                        op=mult)
                nc.vector.tensor_scalar_add(ctx_dst[:], ctx_dst[:], bv_h[h])

            def attn_hp01(qt, fuse_v):
                # heads 0+1 interleaved: both go into one [128, 1024] PSUM
                # tile so exp runs as a single wide op, and the two score
                # matmuls (row groups 0-63 / 64-127) overlap on the PE.
                qs = slice(qt * QT, (qt + 1) * QT)
                Cs = {}
                for h in (0, 1):
                    Cs[h] = ps_c.tile([D_K + 1, QT], f32, name="C", tag="c")
                for kc in range(KC):
                    ks = slice(kc * 128, (kc + 1) * 128)
                    S2 = ps_s.tile([128, 2 * QT], f32, name="S", tag="s")
                    for h in (0, 1):
                        kT_h, qT_h, _ = head_slices(h, qt)
                        nc.tensor.matmul(S2[:, h * QT:(h + 1) * QT],
                                         lhsT=kT_h[:, ks], rhs=qT_h[:, qs])
                    e2 = ep.tile([128, 2 * QT], f16, name="expT")
                    nc.scalar.activation(e2[:], S2[:], Exp, scale=0.125)
                    if fuse_v:
                        v_proj(kc)
                    for h in (0, 1):
                        nc.tensor.matmul(Cs[h][:], lhsT=v_sb[kc][:, h, :],
                                         rhs=e2[:, h * QT:(h + 1) * QT],
                                         start=(kc == 0), stop=(kc == KC - 1))
                for h in (0, 1):
                    normalize(Cs[h], h, qt)

            def attn_h2(qt):
                # head 2: one [128, 1024] scores tile covers two k-chunks;
                # alternate PE row groups via the mirrored b-half
                qs = slice(qt * QT, (qt + 1) * QT)
                C2 = ps_c.tile([D_K + 1, QT], f32, name="C", tag="c")
                for kc2 in range(KC // 2):
                    S2 = ps_s.tile([128, 2 * QT], f32, name="S", tag="s")
                    for i in (0, 1):
                        kc = 2 * kc2 + i
                        rg = slice(64 * i, 64 * i + 64)
                        nc.tensor.matmul(
                            S2[:, i * QT:(i + 1) * QT],
                            lhsT=kT_b[rg, kc * 128:(kc + 1) * 128],
                            rhs=qT_b[rg, qs])
                    e2 = ep.tile([128, 2 * QT], f16, name="expT")
                    nc.scalar.activation(e2[:], S2[:], Exp, scale=0.125)
                    for i in (0, 1):
                        kc = 2 * kc2 + i
                        nc.tensor.matmul(C2[:], lhsT=v_sb[kc][:, 2, :],
                                         rhs=e2[:, i * QT:(i + 1) * QT],
                                         start=(kc == 0), stop=(kc == KC - 1))
                normalize(C2, 2, qt)



            def out_proj(qt, last=False):
                for st in range(QT // 128):
                    r0 = qt * QT + st * 128
                    ws = slice(st * 128, (st + 1) * 128)
                    osb = op.tile([128, D_MODEL], f32, name="osb")
                    for n, ns in enumerate((slice(0, 384), slice(384, 768))):
                        if last:
                            # attention is done: borrow the free S-pool
                            # slots so the tail pipelines
                            po = ps_s.tile([128, 384], f32, name="S",
                                           tag="s")
                        else:
                            po = ps_o.tile([128, 384], f32, name="po",
                                           tag="po")
                        nc.tensor.matmul(po[:], lhsT=ctxT_a[qt][:, ws],
                                         rhs=wo_a[:, ns],
                                         start=True, stop=False)
                        nc.tensor.matmul(po[:], lhsT=ctxT_b[qt][:, ws],
                                         rhs=wo_b[:, ns],
                                         start=False, stop=True)
                        nc.vector.tensor_copy(out=osb[:, ns], in_=po[:])
                    nc.sync.dma_start(out=out[r0:r0 + 128, :], in_=osb[:])

            # software pipeline: attention on heads 0+1 starts as soon as
            # the A-pass projections finish; the B-pass projections (head
            # 2's features), v-projection, and each q-tile's output
            # projection are emitted inside later ACT-bound attention
            # sections so the PE fills its slack instead of serializing.
            attn_hp01(0, fuse_v=True)
            proj_passB(xk_sb, wk_sb, bk_b, kT_b)
            proj_passB(xq_sb, wq_sb, bq_b, qT_b)
            attn_hp01(1, fuse_v=False)
            attn_hp01(2, fuse_v=False)
            attn_hp01(3, fuse_v=False)
            attn_h2(0)
            out_proj(0)
            attn_h2(1)
            out_proj(1)
            attn_h2(2)
            out_proj(2)
            attn_h2(3)
            out_proj(3, last=True)

    nc.compile()
    return nc


def _get_program():
    global _PROGRAM
    if _PROGRAM is None:
        _PROGRAM = _build_program()
    return _PROGRAM


def make_in_maps(query, key, value, Wq, bq, Wk, bk, Wv, bv, Wo, bo):
    """Build the 8 per-core input maps (host-side shard + transpose + cast)."""
    q32 = np.asarray(query, np.float32)
    k32 = np.asarray(key, np.float32)
    v32 = np.asarray(value, np.float32)
    xT = {}
    for b in range(B):
        xT[b] = (np.ascontiguousarray(q32[b].T).astype(np.float16),
                 np.ascontiguousarray(k32[b].T).astype(np.float16),
                 np.ascontiguousarray(v32[b].T).astype(np.float16))
    Wq = np.asarray(Wq, np.float32)
    Wk = np.asarray(Wk, np.float32)
    Wv = np.asarray(Wv, np.float32)
    Wo = np.asarray(Wo, np.float32)
    bq = np.asarray(bq, np.float32)
    bk = np.asarray(bk, np.float32)
    bv = np.asarray(bv, np.float32)
    in_maps = []
    for c in range(N_CORES):
        b, g = divmod(c, G)
        fs = slice(g * GW, (g + 1) * GW)
        xq, xk, xv = xT[b]
        # packed weights [128, WPK]: wq|wk|wv chunks (d-major), wo_a, wo_b
        wps = np.zeros((128, WPK), np.float16)
        for i, W in enumerate((Wk, Wq, Wv)):
            Ws = W[:, fs]
            for d in range(DC):
                wps[:, (i * DC + d) * GW:(i * DC + d + 1) * GW] = \
                    Ws[d * 128:(d + 1) * 128, :].astype(np.float16)
        Wos = Wo[fs, :]
        wps[:, 3 * DC * GW:3 * DC * GW + D_MODEL] = \
            Wos[0:128, :].astype(np.float16)
        wps[0:64, 3 * DC * GW + D_MODEL:WPK] = \
            Wos[128:GW, :].astype(np.float16)
        # packed biases [128, 8] f32
        bps = np.zeros((128, BPK), np.float32)
        bps[:, 0] = bq[fs][0:128]
        bps[0:64, 1] = bq[fs][128:GW]
        bps[:, 2] = bk[fs][0:128]
        bps[0:64, 3] = bk[fs][128:GW]
        for h in range(HPG):
            bps[0:64, 4 + h] = bv[fs][h * 64:(h + 1) * 64]
        in_maps.append({
            "xqT": xq, "xkT": xk, "xvT": xv,
            "wpk": wps, "bpk": bps,
        })
    return in_maps


def combine_outputs(results, bo):
    """Sum the per-core partial outputs into the full [B, S, D] output."""
    bo = np.asarray(bo, np.float32)
    out = np.zeros((B, S, D_MODEL), np.float32)
    for c in range(N_CORES):
        b = c // G
        out[b] += np.asarray(results[c]["out"], np.float32)
    out += bo[None, None, :]
    return out


def kernel(**inputs):
    from concourse.bass_utils import run_bass_kernel_spmd

    nc = _get_program()
    in_maps = make_in_maps(**inputs)
    res = run_bass_kernel_spmd(nc, in_maps, list(range(N_CORES)))
    return combine_outputs(res.results, inputs["bo"])



# revision 7
# speedup vs baseline: 1.0501x; 1.0501x over previous
"""Multi-head attention (B=2, S=2048, D=768, H=12) on 8 trn2 NeuronCores.

Sharding: batch x head-group data/tensor parallel. Core c = b*4+g handles
batch b and heads [3g, 3g+3) (a 192-wide slice of the QKV projections and
the matching 192-row slice of Wo). Each core emits a partial [2048, 768]
fp16 output; the host sums the 4 head-group partials per batch and adds bo.

Device schedule (the kernel is dual-roofline: ~100us of PE streaming and
~97us of ACT exp; the program keeps both pinned):
- DMA order: biases+wk/wq/wv, xk, xq, xv (seq-tile-major groups), wo. The
  exp stream starts as soon as qT is projected (~23us); v is projected
  just-in-time per 128-row seq tile as its chunks land so qt0's ctx never
  waits long.
- Warmup junk matmuls run during the initial DMA so the PE HAM clock gate
  opens (2.4GHz) before the first projection.
- k/q projections interleave passA (features 0:128, heads 0+1) and passB
  (features 128:192, head 2) per arriving input chunk.
- Per q-tile: h01 attention (16 kc iters), then h2 (8 kc-pair iters) with
  the PREVIOUS q-tile's output projection injected one (st, ns) unit per
  iteration into PE slack. Emission is software-pipelined: scores[kc+1]
  is emitted before ctx[kc] so exp[kc+1]'s monotonic semaphore wait never
  covers ctx[kc].
- PSUM: ps_s 2x[128,1024] (scores/exp double buffer, banks 0-3), ps_c
  2x[128,512] (ctx accumulators), ps_w 2x[128,512] (vproj pv / out_proj
  po / passB, one shared tag so the pool stays 2 banks).
- out_proj PSUM->SBUF copies alternate Vector/GpSimd so neither engine
  saturates during h2 phases; output is fp16 (halves the out DMA).
"""

import numpy as np

D_MODEL = 768
NUM_HEADS = 12
D_K = 64
B = 2
S = 2048
N_CORES = 8
G = 4              # head groups (cores per batch)
GW = D_MODEL // G  # 192 features per group = 3 heads
HPG = 3            # heads per group
DC = D_MODEL // 128  # 6 d_model chunks
QT = 512           # q-tile width
NQT = S // QT      # 4
KC = S // 128      # 16 k chunks
ST = S // 128      # 16 seq tiles
WKQV = 3 * DC * GW           # wk|wq|wv packed columns: 3456
WPK = WKQV + 2 * D_MODEL     # + wo_a, wo_b: 4992
BPK = 8            # packed bias columns

_PROGRAM = None


def _build_program():
    from concourse import bacc, tile
    import concourse.mybir as mybir

    f16 = mybir.dt.float16
    f32 = mybir.dt.float32
    Exp = mybir.ActivationFunctionType.Exp
    mult = mybir.AluOpType.mult

    nc = bacc.Bacc("TRN2", target_bir_lowering=False, debug=False,
                   enable_asserts=False)

    xqT = nc.dram_tensor("xqT", [D_MODEL, S], f16, kind="ExternalInput")
    xkT = nc.dram_tensor("xkT", [D_MODEL, S], f16, kind="ExternalInput")
    xvT = nc.dram_tensor("xvT", [D_MODEL, S], f16, kind="ExternalInput")
    wpk = nc.dram_tensor("wpk", [128, WPK], f16, kind="ExternalInput")
    bpk = nc.dram_tensor("bpk", [128, BPK], f32, kind="ExternalInput")
    out = nc.dram_tensor("out", [S, D_MODEL], f16, kind="ExternalOutput")

    with tile.TileContext(nc) as tc:
        with tc.tile_pool(name="const", bufs=1) as cp, \
             tc.tile_pool(name="expp", bufs=4) as ep, \
             tc.tile_pool(name="normp", bufs=2) as np_, \
             tc.tile_pool(name="outp", bufs=3) as op, \
             tc.tile_pool(name="ps_s", bufs=2, space="PSUM") as ps_s, \
             tc.tile_pool(name="ps_c", bufs=2, space="PSUM") as ps_c, \
             tc.tile_pool(name="ps_w", bufs=2, space="PSUM") as ps_w:

            # ---- DMA: biases, then wk|wq|wv, then xk, xq, xv, wo ----
            bps = cp.tile([128, BPK], f32, name="bps")
            nc.sync.dma_start(out=bps[:], in_=bpk[:])
            wps = cp.tile([128, WPK], f16, name="wps")
            nc.sync.dma_start(out=wps[:, 0:WKQV], in_=wpk[:, 0:WKQV])
            wk_sb = [wps[:, d * GW:(d + 1) * GW] for d in range(DC)]
            wq_sb = [wps[:, DC * GW + d * GW:DC * GW + (d + 1) * GW]
                     for d in range(DC)]
            wv_sb = [wps[:, 2 * DC * GW + d * GW:2 * DC * GW + (d + 1) * GW]
                     for d in range(DC)]
            wo_a = wps[:, WKQV:WKQV + D_MODEL]
            wo_b = wps[0:64, WKQV + D_MODEL:WPK]
            bq_a, bq_b = bps[:, 0:1], bps[0:64, 1:2]
            bk_a, bk_b = bps[:, 2:3], bps[0:64, 3:4]
            bv_h = [bps[0:64, 4 + h:5 + h] for h in range(HPG)]

            xk_sb, xq_sb = [], []
            for d in range(DC):
                t = cp.tile([128, S], f16, name=f"xk{d}")
                nc.sync.dma_start(out=t[:], in_=xkT[d * 128:(d + 1) * 128, :])
                xk_sb.append(t)
            for d in range(DC):
                t = cp.tile([128, S], f16, name=f"xq{d}")
                nc.sync.dma_start(out=t[:], in_=xqT[d * 128:(d + 1) * 128, :])
                xq_sb.append(t)
            # xv ships seq-tile-major (4 groups of 4 seq tiles) so the v
            # projection for early k-chunks starts long before xv finishes
            xvg = []
            for g4 in range(4):
                t = cp.tile([128, DC, 512], f16, name=f"xv{g4}")
                cs = slice(g4 * 512, (g4 + 1) * 512)
                for d in range(DC):
                    nc.sync.dma_start(out=t[:, d, :],
                                      in_=xvT[d * 128:(d + 1) * 128, cs])
                xvg.append(t)
            nc.sync.dma_start(out=wps[:, WKQV:WPK], in_=wpk[:, WKQV:WPK])

            # ---- PE warmup: junk matmuls keep the PE busy during the
            # initial DMA so the HAM clock gate opens before the real
            # projections. Sources are never-written SBUF (values are
            # irrelevant; the PSUM banks are overwritten with start=True
            # later).
            junk = cp.tile([128, QT], f16, name="junk")
            nc.vector.memset(junk[:], 0.5)
            wup = [ps_s.tile([128, 2 * QT], f32, name="S", tag="s")
                   for _ in range(2)]
            for i in range(14):
                nc.tensor.matmul(wup[i % 2][:, 0:QT], lhsT=junk[:, 0:128],
                                 rhs=junk[:], start=True, stop=True)
            # dummy exp: pulls the ACT exp table load into the DMA shadow
            escr = cp.tile([128, BPK], f16, name="escr")
            nc.scalar.activation(escr[:], bps[:], Exp, scale=0.001)

            # ---- k/q projections: passA (128 feats) + passB (64 feats)
            # interleaved per input chunk so each chunk is consumed once as
            # it arrives ----
            qT_a = cp.tile([128, S], f16, name="qT_a")
            qT_b = cp.tile([128, S], f16, name="qT_b")
            kT_a = cp.tile([128, S], f16, name="kT_a")
            kT_b = cp.tile([128, S], f16, name="kT_b")

            def proj(x_sb, w_sb, b_a, b_b, dst_a, dst_b):
                pjA = [ps_s.tile([128, 2 * QT], f32, name="S", tag="s")
                       for _ in range(2)]
                pjB = [ps_c.tile([128, QT], f32, name="C", tag="c")
                       for _ in range(2)]
                pjB += [ps_w.tile([128, QT], f32, name="W", tag="w")
                        for _ in range(2)]
                for d in range(DC):
                    for j2 in range(2):
                        for n in range(2):
                            cs = slice(j2 * 1024 + n * QT,
                                       j2 * 1024 + (n + 1) * QT)
                            nc.tensor.matmul(
                                pjA[j2][:, n * QT:(n + 1) * QT],
                                lhsT=w_sb[d][:, 0:128], rhs=x_sb[d][:, cs],
                                start=(d == 0), stop=(d == DC - 1))
                    for n4 in range(4):
                        cs = slice(n4 * QT, (n4 + 1) * QT)
                        nc.tensor.matmul(pjB[n4][0:64, :],
                                         lhsT=w_sb[d][:, 128:GW],
                                         rhs=x_sb[d][:, cs],
                                         start=(d == 0), stop=(d == DC - 1))
                for j2 in range(2):
                    js = slice(j2 * 1024, (j2 + 1) * 1024)
                    nc.vector.tensor_scalar_add(dst_a[:, js], pjA[j2][:], b_a)
                for n4 in range(4):
                    cs = slice(n4 * QT, (n4 + 1) * QT)
                    nc.vector.tensor_scalar_add(dst_b[0:64, cs],
                                                pjB[n4][0:64, :], b_b)
                # mirror the 64-row b-half into partitions 64-127 so head-2
                # score matmuls can alternate PE row groups
                nc.sync.dma_start(out=dst_b[64:128, :], in_=dst_b[0:64, :])

            proj(xk_sb, wk_sb, bk_a, bk_b, kT_a, kT_b)
            proj(xq_sb, wq_sb, bq_a, bq_b, qT_a, qT_b)

            # ---- v projection: just-in-time per seq tile (only emitted
            # inside qt0's h01 loop) ----
            v_sb = [None] * ST

            def v_proj(st):
                g4, i4 = st // 4, st % 4
                pv = ps_w.tile([128, QT], f32, name="W", tag="w")
                for d in range(DC):
                    nc.tensor.matmul(pv[:, 0:GW],
                                     lhsT=xvg[g4][:, d, i4 * 128:(i4 + 1) * 128],
                                     rhs=wv_sb[d][:],
                                     start=(d == 0), stop=(d == DC - 1))
                vt = cp.tile([128, HPG, D_K + 1], f16, name=f"vsb{st}")
                nc.vector.tensor_copy(out=vt[:, :, 0:D_K],
                                      in_=pv[:, 0:GW].rearrange(
                                          "p (h w) -> p h w", h=HPG))
                nc.vector.memset(vt[:, :, D_K:D_K + 1], 1.0)
                v_sb[st] = vt

            # ---- attention state ----
            ctxT_a = [cp.tile([128, QT], f16, name=f"ctxTa{j}")
                      for j in range(NQT)]
            ctxT_b = [cp.tile([64, QT], f16, name=f"ctxTb{j}")
                      for j in range(NQT)]

            def normalize(C, h, qt):
                # ctxT = C[0:64] * (1/denom) + bv. reciprocal_approx_fast
                # must read SBUF, so stage the denominator row first.
                ctx_dst = (ctxT_a[qt][0:64] if h == 0 else
                           ctxT_a[qt][64:128] if h == 1 else
                           ctxT_b[qt][0:64])
                den = np_.tile([1, QT], f32, name="den")
                nc.vector.tensor_copy(out=den[:], in_=C[D_K:D_K + 1, :])
                r = np_.tile([1, QT], f32, name="r")
                nc.vector.reciprocal_approx_fast(out=r[:], in_=den[:])
                bc = np_.tile([128, QT], f32, name="bc")
                nc.gpsimd.partition_broadcast(bc[:], r[:])
                base = 64 if h == 1 else 0
                nc.vector.tensor_tensor(out=ctx_dst[:],
                                        in0=C[0:D_K, :],
                                        in1=bc[base:base + D_K, :],
                                        op=mult)
                nc.vector.tensor_scalar_add(ctx_dst[:], ctx_dst[:], bv_h[h])

            osb = [None] * (ST // 4)

            def out_unit(qt, u):
                # one (st, ns) unit of q-tile qt's output projection
                st, ns_i = u // 2, u % 2
                ws = slice(st * 128, (st + 1) * 128)
                ns = slice(ns_i * 384, (ns_i + 1) * 384)
                if ns_i == 0:
                    osb[st] = op.tile([128, D_MODEL], f16, name="osb")
                po = ps_w.tile([128, QT], f32, name="W", tag="w")
                nc.tensor.matmul(po[:, 0:384], lhsT=ctxT_a[qt][:, ws],
                                 rhs=wo_a[:, ns], start=True, stop=False)
                nc.tensor.matmul(po[:, 0:384], lhsT=ctxT_b[qt][:, ws],
                                 rhs=wo_b[:, ns], start=False, stop=True)
                nc.vector.tensor_copy(out=osb[st][:, ns], in_=po[:, 0:384])
                if ns_i == 1:
                    r0 = qt * QT + st * 128
                    nc.sync.dma_start(out=out[r0:r0 + 128, :], in_=osb[st][:])

            def attn_hp01(qt):
                # heads 0+1: row-group-paired score matmuls into one
                # [128,1024] PSUM tile, exp as a single wide op. Emission is
                # software-pipelined: scores[kc+1] precedes ctx[kc].
                qs = slice(qt * QT, (qt + 1) * QT)
                Cs = {h: ps_c.tile([128, QT], f32, name="C", tag="c")
                      for h in (0, 1)}
                S2s = [None] * KC

                def scores(kc):
                    ks = slice(kc * 128, (kc + 1) * 128)
                    S2 = ps_s.tile([128, 2 * QT], f32, name="S", tag="s")
                    nc.tensor.matmul(S2[:, 0:QT], lhsT=kT_a[0:64, ks],
                                     rhs=qT_a[0:64, qs])
                    nc.tensor.matmul(S2[:, QT:2 * QT], lhsT=kT_a[64:128, ks],
                                     rhs=qT_a[64:128, qs])
                    S2s[kc] = S2

                scores(0)
                for kc in range(KC):
                    e2 = ep.tile([128, 2 * QT], f16, name="expT")
                    nc.scalar.activation(e2[:], S2s[kc][:], Exp, scale=0.125)
                    if kc + 1 < KC:
                        scores(kc + 1)
                    if qt == 0:
                        v_proj(kc)
                    for h in (0, 1):
                        nc.tensor.matmul(Cs[h][0:D_K + 1, :],
                                         lhsT=v_sb[kc][:, h, :],
                                         rhs=e2[:, h * QT:(h + 1) * QT],
                                         start=(kc == 0), stop=(kc == KC - 1))
                for h in (0, 1):
                    normalize(Cs[h], h, qt)

            def attn_h2(qt, oqt):
                # head 2: one [128,1024] scores tile covers two k-chunks via
                # the mirrored b-half; out_proj units for q-tile oqt are
                # injected one per iteration.
                qs = slice(qt * QT, (qt + 1) * QT)
                C2 = ps_c.tile([128, QT], f32, name="C", tag="c")
                S2s = [None] * (KC // 2)

                def scores2(kc2):
                    S2 = ps_s.tile([128, 2 * QT], f32, name="S", tag="s")
                    for i in (0, 1):
                        kc = 2 * kc2 + i
                        rg = slice(64 * i, 64 * i + 64)
                        nc.tensor.matmul(
                            S2[:, i * QT:(i + 1) * QT],
                            lhsT=kT_b[rg, kc * 128:(kc + 1) * 128],
                            rhs=qT_b[rg, qs])
                    S2s[kc2] = S2

                scores2(0)
                for kc2 in range(KC // 2):
                    e2 = ep.tile([128, 2 * QT], f16, name="expT")
                    nc.scalar.activation(e2[:], S2s[kc2][:], Exp, scale=0.125)
                    if kc2 + 1 < KC // 2:
                        scores2(kc2 + 1)
                    for i in (0, 1):
                        kc = 2 * kc2 + i
                        nc.tensor.matmul(C2[0:D_K + 1, :],
                                         lhsT=v_sb[kc][:, 2, :],
                                         rhs=e2[:, i * QT:(i + 1) * QT],
                                         start=(kc == 0), stop=(kc == KC - 1))
                    if oqt is not None:
                        out_unit(oqt, kc2)
                normalize(C2, 2, qt)

            for qt in range(NQT):
                attn_hp01(qt)
                attn_h2(qt, qt - 1 if qt > 0 else None)
            for u in range(8):
                out_unit(NQT - 1, u)

    nc.compile()
    return nc


def _get_program():
    global _PROGRAM
    if _PROGRAM is None:
        _PROGRAM = _build_program()
    return _PROGRAM


def make_in_maps(query, key, value, Wq, bq, Wk, bk, Wv, bv, Wo, bo):
    """Build the 8 per-core input maps (host-side shard + transpose + cast)."""
    q32 = np.asarray(query, np.float32)
    k32 = np.asarray(key, np.float32)
    v32 = np.asarray(value, np.float32)
    xT = {}
    for b in range(B):
        xT[b] = (np.ascontiguousarray(q32[b].T).astype(np.float16),
                 np.ascontiguousarray(k32[b].T).astype(np.float16),
                 np.ascontiguousarray(v32[b].T).astype(np.float16))
    Wq = np.asarray(Wq, np.float32)
    Wk = np.asarray(Wk, np.float32)
    Wv = np.asarray(Wv, np.float32)
    Wo = np.asarray(Wo, np.float32)
    bq = np.asarray(bq, np.float32)
    bk = np.asarray(bk, np.float32)
    bv = np.asarray(bv, np.float32)
    in_maps = []
    for c in range(N_CORES):
        b, g = divmod(c, G)
        fs = slice(g * GW, (g + 1) * GW)
        xq, xk, xv = xT[b]
        # packed weights [128, WPK]: wk|wq|wv chunks (d-major), wo_a, wo_b
        wps = np.zeros((128, WPK), np.float16)
        for i, W in enumerate((Wk, Wq, Wv)):
            Ws = W[:, fs]
            for d in range(DC):
                wps[:, (i * DC + d) * GW:(i * DC + d + 1) * GW] = \
                    Ws[d * 128:(d + 1) * 128, :].astype(np.float16)
        Wos = Wo[fs, :]
        wps[:, WKQV:WKQV + D_MODEL] = Wos[0:128, :].astype(np.float16)
        wps[0:64, WKQV + D_MODEL:WPK] = Wos[128:GW, :].astype(np.float16)
        # packed biases [128, 8] f32
        bps = np.zeros((128, BPK), np.float32)
        bps[:, 0] = bq[fs][0:128]
        bps[0:64, 1] = bq[fs][128:GW]
        bps[:, 2] = bk[fs][0:128]
        bps[0:64, 3] = bk[fs][128:GW]
        for h in range(HPG):
            bps[0:64, 4 + h] = bv[fs][h * 64:(h + 1) * 64]
        in_maps.append({
            "xqT": xq, "xkT": xk, "xvT": xv,
            "wpk": wps, "bpk": bps,
        })
    return in_maps


def combine_outputs(results, bo):
    """Sum the per-core partial outputs into the full [B, S, D] output."""
    bo = np.asarray(bo, np.float32)
    out = np.zeros((B, S, D_MODEL), np.float32)
    for c in range(N_CORES):
        b = c // G
        out[b] += np.asarray(results[c]["out"], np.float32)
    out += bo[None, None, :]
    return out


def kernel(**inputs):
    from concourse.bass_utils import run_bass_kernel_spmd

    nc = _get_program()
    in_maps = make_in_maps(**inputs)
    res = run_bass_kernel_spmd(nc, in_maps, list(range(N_CORES)))
    return combine_outputs(res.results, inputs["bo"])


# revision 11
# speedup vs baseline: 1.0622x; 1.0115x over previous
"""Multi-head attention (B=2, S=2048, D=768, H=12) on 8 trn2 NeuronCores.

Sharding: batch x head-group data/tensor parallel. Core c = b*4+g handles
batch b and heads [3g, 3g+3) (a 192-wide slice of the QKV projections and
the matching 192-row slice of Wo). Each core emits a partial [2048, 768]
fp16 output; the host sums the 4 head-group partials per batch and adds bo.

Device schedule. The kernel is dual-roofline (~100us PE streaming, ~97us
ACT exp); the program starts the exp stream as early as the DMA allows and
keeps both engines pinned:
- DMA order: biases, wk|wq|wv, xk, xq, xv, wo. xv is host-rearranged into
  seq-tile-major groups with 3KB contiguous lines (the DMA engines are
  descriptor-bound at ~10ns/line, so thin lines are slow) and the v
  projection runs just-in-time per seq tile inside qt0's attention loop.
- Only the work needed for the first exp runs before attention: k passA
  (all columns) and q passA columns 0:1024. The rest (q passA cols
  1024:2048, k/q passB for head 2) is deferred into qt1-3's PE slack as
  six-matmul units cycling through the 2-bank "w" PSUM pool.
- Warmup junk matmuls (in the "w" pool, so no WAW against the projection
  accumulators) open the PE HAM clock gate during the initial DMA.
- Phases: h01(qt0..3), then h2(qt)+out_proj(qt-1) with one (st, ns) unit
  injected per h2 iteration; out_proj(qt3) forms the tail with copies
  alternating Scalar/Vector. Emission is software-pipelined: scores[kc+1]
  precedes ctx[kc] so exp[kc+1]'s monotonic semaphore wait never covers
  ctx[kc].
- PSUM: ps_s 2x[128,1024] scores/exp double buffer; ps_c 2x[128,512] ctx
  accumulators (normalize of qt overlaps qt+1's attention); ps_w
  2x[128,512] shared by warmup/vproj/deferred-proj/out_proj.
- ctxT_b is mirrored into partitions 64:127 so consecutive out_proj
  b-matmuls alternate PE row groups and overlap.
"""

import numpy as np

D_MODEL = 768
NUM_HEADS = 12
D_K = 64
B = 2
S = 2048
N_CORES = 8
G = 4              # head groups (cores per batch)
GW = D_MODEL // G  # 192 features per group = 3 heads
HPG = 3            # heads per group
DC = D_MODEL // 128  # 6 d_model chunks
QT = 512           # q-tile width
NQT = S // QT      # 4
KC = S // 128      # 16 k chunks
ST = S // 128      # 16 seq tiles
WKQV = 3 * DC * GW           # wk|wq|wv packed columns: 3456
WPK = WKQV + 2 * D_MODEL     # + wo_a, wo_b: 4992
BPK = 8            # packed bias columns
XVG = 8            # xv seq-tile-pair groups
XVW = 2 * DC * 128  # columns per xv group: 1536

_PROGRAM = None


def _build_program():
    from concourse import bacc, tile
    import concourse.mybir as mybir

    f16 = mybir.dt.float16
    f32 = mybir.dt.float32
    Exp = mybir.ActivationFunctionType.Exp
    mult = mybir.AluOpType.mult

    nc = bacc.Bacc("TRN2", target_bir_lowering=False, debug=False,
                   enable_asserts=False)

    xqT = nc.dram_tensor("xqT", [D_MODEL, S], f16, kind="ExternalInput")
    xkT = nc.dram_tensor("xkT", [D_MODEL, S], f16, kind="ExternalInput")
    xvR = nc.dram_tensor("xvR", [128, XVG * XVW], f16, kind="ExternalInput")
    wpk = nc.dram_tensor("wpk", [128, WPK], f16, kind="ExternalInput")
    bpk = nc.dram_tensor("bpk", [128, BPK], f32, kind="ExternalInput")
    out = nc.dram_tensor("out", [S, D_MODEL], f16, kind="ExternalOutput")

    with tile.TileContext(nc) as tc:
        with tc.tile_pool(name="const", bufs=1) as cp, \
             tc.tile_pool(name="expp", bufs=4) as ep, \
             tc.tile_pool(name="normp", bufs=2) as np_, \
             tc.tile_pool(name="outp", bufs=3) as op, \
             tc.tile_pool(name="ps_s", bufs=2, space="PSUM") as ps_s, \
             tc.tile_pool(name="ps_c", bufs=2, space="PSUM") as ps_c, \
             tc.tile_pool(name="ps_w", bufs=2, space="PSUM") as ps_w:

            # ---- DMA: biases, wk|wq|wv, xk, xq, xv (st-pair major), wo ----
            bps = cp.tile([128, BPK], f32, name="bps")
            nc.sync.dma_start(out=bps[:], in_=bpk[:])
            wps = cp.tile([128, WPK], f16, name="wps")
            nc.sync.dma_start(out=wps[:, 0:WKQV], in_=wpk[:, 0:WKQV])
            wk_sb = [wps[:, d * GW:(d + 1) * GW] for d in range(DC)]
            wq_sb = [wps[:, DC * GW + d * GW:DC * GW + (d + 1) * GW]
                     for d in range(DC)]
            wv_sb = [wps[:, 2 * DC * GW + d * GW:2 * DC * GW + (d + 1) * GW]
                     for d in range(DC)]
            wo_a = wps[:, WKQV:WKQV + D_MODEL]
            wo_b = wps[0:64, WKQV + D_MODEL:WPK]
            bq_a, bq_b = bps[:, 0:1], bps[0:64, 1:2]
            bk_a, bk_b = bps[:, 2:3], bps[0:64, 3:4]
            bv_h = [bps[0:64, 4 + h:5 + h] for h in range(HPG)]

            xk_sb, xq_sb = [], []
            for d in range(DC):
                t = cp.tile([128, S], f16, name=f"xk{d}")
                nc.sync.dma_start(out=t[:], in_=xkT[d * 128:(d + 1) * 128, :])
                xk_sb.append(t)
            for d in range(DC):
                t = cp.tile([128, S], f16, name=f"xq{d}")
                nc.sync.dma_start(out=t[:], in_=xqT[d * 128:(d + 1) * 128, :])
                xq_sb.append(t)
            xvg = []
            for g8 in range(XVG):
                t = cp.tile([128, 2, DC, 128], f16, name=f"xv{g8}")
                nc.sync.dma_start(
                    out=t[:],
                    in_=xvR[:, g8 * XVW:(g8 + 1) * XVW].rearrange(
                        "p (s d c) -> p s d c", s=2, d=DC))
                xvg.append(t)
            nc.sync.dma_start(out=wps[:, WKQV:WPK], in_=wpk[:, WKQV:WPK])
            # wo_b mirrored into partitions 64:127 so out_proj b-matmuls can
            # alternate PE row groups (lhsT/rhs must share a base partition)
            wo_bm = cp.tile([128, D_MODEL], f16, name="wo_bm")
            nc.sync.dma_start(out=wo_bm[0:64, :],
                              in_=wpk[0:64, WKQV + D_MODEL:WPK])
            nc.sync.dma_start(out=wo_bm[64:128, :],
                              in_=wpk[0:64, WKQV + D_MODEL:WPK])

            # ---- PE warmup in the "w" pool (no WAW against the "s"-pool
            # projection accumulators); sources are a zeroed scratch tile ----
            junk = cp.tile([128, 256], f16, name="junk")
            nc.vector.memset(junk[:], 0.5)
            wupt = [ps_w.tile([128, QT], f32, name="W", tag="w")
                    for _ in range(2)]
            for i in range(8):
                nc.tensor.matmul(wupt[i % 2][:, 0:256], lhsT=junk[:, 0:128],
                                 rhs=junk[:], start=True, stop=True)
            # dummy exp pulls the ACT exp-table load into the DMA shadow
            escr = cp.tile([128, BPK], f16, name="escr")
            nc.scalar.activation(escr[:], bps[:], Exp, scale=0.001)

            # ---- pre-attention projections: k passA (all columns), q
            # passA columns 0:1024 ----
            qT_a = cp.tile([128, S], f16, name="qT_a")
            qT_b = cp.tile([128, S], f16, name="qT_b")
            kT_a = cp.tile([128, S], f16, name="kT_a")
            kT_b = cp.tile([128, S], f16, name="kT_b")

            pjK = [ps_s.tile([128, 2 * QT], f32, name="S", tag="s")
                   for _ in range(2)]
            for d in range(DC):
                for j2 in range(2):
                    for n in range(2):
                        cs = slice(j2 * 1024 + n * QT,
                                   j2 * 1024 + (n + 1) * QT)
                        nc.tensor.matmul(
                            pjK[j2][:, n * QT:(n + 1) * QT],
                            lhsT=wk_sb[d][:, 0:128], rhs=xk_sb[d][:, cs],
                            start=(d == 0), stop=(d == DC - 1))
            for j2 in range(2):
                js = slice(j2 * 1024, (j2 + 1) * 1024)
                nc.vector.tensor_scalar_add(kT_a[:, js], pjK[j2][:], bk_a)

            pjQ = ps_s.tile([128, 2 * QT], f32, name="S", tag="s")
            for d in range(DC):
                for n in range(2):
                    nc.tensor.matmul(
                        pjQ[:, n * QT:(n + 1) * QT],
                        lhsT=wq_sb[d][:, 0:128], rhs=xq_sb[d][:, n * QT:
                                                              (n + 1) * QT],
                        start=(d == 0), stop=(d == DC - 1))
            nc.vector.tensor_scalar_add(qT_a[:, 0:1024], pjQ[:], bq_a)

            # ---- deferred projection units (injected into qt1-3's h01
            # slack): q passA cols 1024:2048, then k/q passB (head 2).
            # Each unit = 6 accumulating matmuls into a "w" tile + a bias
            # add; a unit occupies one PSUM bank for ~3 iterations. ----
            fillers = []

            def add_unit(xsel, wsel, w_lo, w_hi, b, dst, cs, last_of_pass,
                         mirror):
                state = {}
                rows = w_hi - w_lo

                def mk(d):
                    def emit():
                        if d == 0:
                            state["pj"] = ps_w.tile([128, QT], f32,
                                                    name="W", tag="w")
                        nc.tensor.matmul(
                            state["pj"][0:rows, :],
                            lhsT=wsel[d][:, w_lo:w_hi], rhs=xsel[d][:, cs],
                            start=(d == 0), stop=(d == DC - 1))
                        if d == DC - 1:
                            nc.vector.tensor_scalar_add(
                                dst[0:rows, cs], state["pj"][0:rows, :], b)
                            if last_of_pass and mirror:
                                nc.sync.dma_start(out=dst[64:128, :],
                                                  in_=dst[0:64, :])
                    return emit
                for d in range(DC):
                    fillers.append(mk(d))

            for n in range(2):  # q passA cols 1024:2048
                add_unit(xq_sb, wq_sb, 0, 128, bq_a, qT_a,
                         slice(1024 + n * QT, 1024 + (n + 1) * QT),
                         False, False)
            for n4 in range(4):  # k passB
                add_unit(xk_sb, wk_sb, 128, GW, bk_b, kT_b,
                         slice(n4 * QT, (n4 + 1) * QT), n4 == 3, True)
            for n4 in range(4):  # q passB
                add_unit(xq_sb, wq_sb, 128, GW, bq_b, qT_b,
                         slice(n4 * QT, (n4 + 1) * QT), n4 == 3, True)

            # ---- v projection: just-in-time per seq tile inside qt0 ----
            v_sb = [None] * ST

            def v_proj(st):
                g8, s2 = st // 2, st % 2
                pv = ps_w.tile([128, QT], f32, name="W", tag="w")
                for d in range(DC):
                    nc.tensor.matmul(pv[:, 0:GW],
                                     lhsT=xvg[g8][:, s2, d, :],
                                     rhs=wv_sb[d][:],
                                     start=(d == 0), stop=(d == DC - 1))
                vt = cp.tile([128, HPG, D_K + 1], f16, name=f"vsb{st}")
                nc.vector.tensor_copy(out=vt[:, :, 0:D_K],
                                      in_=pv[:, 0:GW].rearrange(
                                          "p (h w) -> p h w", h=HPG))
                nc.vector.memset(vt[:, :, D_K:D_K + 1], 1.0)
                v_sb[st] = vt

            # ---- attention state ----
            ctxT_a = [cp.tile([128, QT], f16, name=f"ctxTa{j}")
                      for j in range(NQT)]
            ctxT_b = [cp.tile([128, QT], f16, name=f"ctxTb{j}")
                      for j in range(NQT)]

            def normalize(C, h, qt):
                # ctxT = C[0:64] * (1/denom) + bv. reciprocal_approx_fast
                # must read SBUF, so stage the denominator row first.
                ctx_dst = (ctxT_a[qt][0:64] if h == 0 else
                           ctxT_a[qt][64:128] if h == 1 else
                           ctxT_b[qt][0:64])
                den = np_.tile([1, QT], f32, name="den")
                nc.vector.tensor_copy(out=den[:], in_=C[D_K:D_K + 1, :])
                r = np_.tile([1, QT], f32, name="r")
                nc.vector.reciprocal_approx_fast(out=r[:], in_=den[:])
                bc = np_.tile([128, QT], f32, name="bc")
                nc.gpsimd.partition_broadcast(bc[:], r[:])
                base = 64 if h == 1 else 0
                nc.vector.tensor_tensor(out=ctx_dst[:],
                                        in0=C[0:D_K, :],
                                        in1=bc[base:base + D_K, :],
                                        op=mult)
                nc.vector.tensor_scalar_add(ctx_dst[:], ctx_dst[:], bv_h[h])
                if h == 2:
                    # mirror so out_proj b-matmuls can alternate row groups
                    nc.sync.dma_start(out=ctxT_b[qt][64:128, :],
                                      in_=ctxT_b[qt][0:64, :])

            osb = [None] * (ST // 4)

            def out_unit(qt, u, tail=False):
                # one (st, ns) unit of q-tile qt's output projection
                st, ns_i = u // 2, u % 2
                ws = slice(st * 128, (st + 1) * 128)
                ns = slice(ns_i * 384, (ns_i + 1) * 384)
                if ns_i == 0:
                    osb[st] = op.tile([128, D_MODEL], f16, name="osb")
                po = ps_w.tile([128, QT], f32, name="W", tag="w")
                nc.tensor.matmul(po[:, 0:384], lhsT=ctxT_a[qt][:, ws],
                                 rhs=wo_a[:, ns], start=True, stop=False)
                rb = slice(0, 64) if u % 2 == 0 else slice(64, 128)
                nc.tensor.matmul(po[:, 0:384], lhsT=ctxT_b[qt][rb, ws],
                                 rhs=wo_bm[rb, ns], start=False, stop=True)
                if tail and u % 2 == 1:
                    nc.scalar.copy(osb[st][:, ns], po[:, 0:384])
                else:
                    nc.vector.tensor_copy(out=osb[st][:, ns],
                                          in_=po[:, 0:384])
                if ns_i == 1:
                    r0 = qt * QT + st * 128
                    nc.sync.dma_start(out=out[r0:r0 + 128, :], in_=osb[st][:])

            def attn_hp01(qt):
                # heads 0+1: row-group-paired score matmuls into one
                # [128,1024] PSUM tile, exp as a single wide op.
                qs = slice(qt * QT, (qt + 1) * QT)
                Cs = {h: ps_c.tile([128, QT], f32, name="C", tag="c")
                      for h in (0, 1)}
                S2s = [None] * KC

                def scores(kc):
                    ks = slice(kc * 128, (kc + 1) * 128)
                    S2 = ps_s.tile([128, 2 * QT], f32, name="S", tag="s")
                    nc.tensor.matmul(S2[:, 0:QT], lhsT=kT_a[0:64, ks],
                                     rhs=qT_a[0:64, qs])
                    nc.tensor.matmul(S2[:, QT:2 * QT], lhsT=kT_a[64:128, ks],
                                     rhs=qT_a[64:128, qs])
                    S2s[kc] = S2

                scores(0)
                for kc in range(KC):
                    e2 = ep.tile([128, 2 * QT], f16, name="expT")
                    nc.scalar.activation(e2[:], S2s[kc][:], Exp, scale=0.125)
                    if kc + 1 < KC:
                        scores(kc + 1)
                    if qt == 0:
                        v_proj(kc)
                    else:
                        for _ in range(2):
                            if fillers:
                                fillers.pop(0)()
                    for h in (0, 1):
                        nc.tensor.matmul(Cs[h][0:D_K + 1, :],
                                         lhsT=v_sb[kc][:, h, :],
                                         rhs=e2[:, h * QT:(h + 1) * QT],
                                         start=(kc == 0), stop=(kc == KC - 1))
                for h in (0, 1):
                    normalize(Cs[h], h, qt)

            def attn_h2(qt, oqt):
                # head 2: one [128,1024] scores tile covers two k-chunks via
                # the mirrored b-half; out_proj units for q-tile oqt are
                # injected one per iteration.
                qs = slice(qt * QT, (qt + 1) * QT)
                C2 = ps_c.tile([128, QT], f32, name="C", tag="c")
                S2s = [None] * (KC // 2)

                def scores2(kc2):
                    S2 = ps_s.tile([128, 2 * QT], f32, name="S", tag="s")
                    for i in (0, 1):
                        kc = 2 * kc2 + i
                        rg = slice(64 * i, 64 * i + 64)
                        nc.tensor.matmul(
                            S2[:, i * QT:(i + 1) * QT],
                            lhsT=kT_b[rg, kc * 128:(kc + 1) * 128],
                            rhs=qT_b[rg, qs])
                    S2s[kc2] = S2

                scores2(0)
                for kc2 in range(KC // 2):
                    e2 = ep.tile([128, 2 * QT], f16, name="expT")
                    nc.scalar.activation(e2[:], S2s[kc2][:], Exp, scale=0.125)
                    if kc2 + 1 < KC // 2:
                        scores2(kc2 + 1)
                    for i in (0, 1):
                        kc = 2 * kc2 + i
                        nc.tensor.matmul(C2[0:D_K + 1, :],
                                         lhsT=v_sb[kc][:, 2, :],
                                         rhs=e2[:, i * QT:(i + 1) * QT],
                                         start=(kc == 0), stop=(kc == KC - 1))
                    if oqt is not None:
                        out_unit(oqt, kc2)
                normalize(C2, 2, qt)

            for qt in range(NQT):
                attn_hp01(qt)
            for qt in range(NQT):
                attn_h2(qt, qt - 1 if qt > 0 else None)
            for u in range(8):
                out_unit(NQT - 1, u, tail=True)

    nc.compile()
    return nc


def _get_program():
    global _PROGRAM
    if _PROGRAM is None:
        _PROGRAM = _build_program()
    return _PROGRAM


def make_in_maps(query, key, value, Wq, bq, Wk, bk, Wv, bv, Wo, bo):
    """Build the 8 per-core input maps (host-side shard + transpose + cast)."""
    q32 = np.asarray(query, np.float32)
    k32 = np.asarray(key, np.float32)
    v32 = np.asarray(value, np.float32)
    xT = {}
    for b in range(B):
        xq = np.ascontiguousarray(q32[b].T).astype(np.float16)
        xk = np.ascontiguousarray(k32[b].T).astype(np.float16)
        xvT = v32[b].T.astype(np.float16)  # [768, 2048]
        # seq-tile-major rearrangement: xvR[p, st*768 + d*128 + c]
        #   = xvT[d*128 + p, st*128 + c]  -> contiguous 1536-col groups
        xvR = np.ascontiguousarray(
            xvT.reshape(DC, 128, ST, 128).transpose(1, 2, 0, 3)
        ).reshape(128, ST * DC * 128)
        xT[b] = (xq, xk, xvR)
    Wq = np.asarray(Wq, np.float32)
    Wk = np.asarray(Wk, np.float32)
    Wv = np.asarray(Wv, np.float32)
    Wo = np.asarray(Wo, np.float32)
    bq = np.asarray(bq, np.float32)
    bk = np.asarray(bk, np.float32)
    bv = np.asarray(bv, np.float32)
    in_maps = []
    for c in range(N_CORES):
        b, g = divmod(c, G)
        fs = slice(g * GW, (g + 1) * GW)
        xq, xk, xvR = xT[b]
        # packed weights [128, WPK]: wk|wq|wv chunks (d-major), wo_a, wo_b
        wps = np.zeros((128, WPK), np.float16)
        for i, W in enumerate((Wk, Wq, Wv)):
            Ws = W[:, fs]
            for d in range(DC):
                wps[:, (i * DC + d) * GW:(i * DC + d + 1) * GW] = \
                    Ws[d * 128:(d + 1) * 128, :].astype(np.float16)
        Wos = Wo[fs, :]
        wps[:, WKQV:WKQV + D_MODEL] = Wos[0:128, :].astype(np.float16)
        wps[0:64, WKQV + D_MODEL:WPK] = Wos[128:GW, :].astype(np.float16)
        # packed biases [128, 8] f32
        bps = np.zeros((128, BPK), np.float32)
        bps[:, 0] = bq[fs][0:128]
        bps[0:64, 1] = bq[fs][128:GW]
        bps[:, 2] = bk[fs][0:128]
        bps[0:64, 3] = bk[fs][128:GW]
        for h in range(HPG):
            bps[0:64, 4 + h] = bv[fs][h * 64:(h + 1) * 64]
        in_maps.append({
            "xqT": xq, "xkT": xk, "xvR": xvR,
            "wpk": wps, "bpk": bps,
        })
    return in_maps


def combine_outputs(results, bo):
    """Sum the per-core partial outputs into the full [B, S, D] output."""
    bo = np.asarray(bo, np.float32)
    out = np.zeros((B, S, D_MODEL), np.float32)
    for c in range(N_CORES):
        b = c // G
        out[b] += np.asarray(results[c]["out"], np.float32)
    out += bo[None, None, :]
    return out


def kernel(**inputs):
    from concourse.bass_utils import run_bass_kernel_spmd

    nc = _get_program()
    in_maps = make_in_maps(**inputs)
    res = run_bass_kernel_spmd(nc, in_maps, list(range(N_CORES)))
    return combine_outputs(res.results, inputs["bo"])


# revision 16
# speedup vs baseline: 1.0819x; 1.0186x over previous
"""Multi-head attention (B=2, S=2048, D=768, H=12) on 8 trn2 NeuronCores.

Sharding: batch x head-group data/tensor parallel. Core c = b*4+g handles
batch b and heads [3g, 3g+3) (a 192-wide slice of the QKV projections and
the matching 192-row slice of Wo). Each core emits a partial [2048, 768]
fp16 output; the host sums the 4 head-group partials per batch and adds bo.

Device schedule. The kernel is dual-roofline (~100us PE streaming, ~97us
ACT exp). The DMA engines are descriptor-bound (~10ns per partition line,
~1.3us per 128-line transfer regardless of bytes), so all inputs are
host-rearranged into few transfers with large contiguous lines:
  wA   [128, 2312]  biases(f16) | wk | wq          (4.6KB lines)
  xkP  [128, 12288] two 6KB-line halves, each [6 chunks x 1024 seq]
  xqP  [128, 12288] same layout as xkP
  wB   [128, 2688]  wv | wo_a | wo_b mirrored      (5.4KB lines)
  xvR  4 x [128, 3072] seq-tile-quad groups        (6KB lines)
DMA order: wA, xk, xq-half0, wB, xv-g0, xq-half1, xv-g1..3 — the first
exp fires right after xq-half0 lands (~26us) and the v projection runs
just-in-time per seq tile inside qt0's attention loop.

Only the work needed for the first exp runs before attention: k passA and
q passA columns 0:1024. The rest (q passA cols 1024:2048, k/q passB for
head 2) is deferred into qt1-3's PE slack as six-matmul units cycling
through the 2-bank "w" PSUM pool. Warmup junk matmuls (also "w") open the
PE HAM clock gate during the initial DMA.

Phases: h01(qt0..3), then h2(qt)+out_proj(qt-1) one (st, ns) unit per h2
iteration; out_proj(qt3) is the tail (borrowing the idle score banks so
four units are in flight, copies alternating Scalar/Vector). Emission is
software-pipelined: scores[kc+1] precedes ctx[kc] so exp[kc+1]'s
monotonic semaphore wait never covers ctx[kc]. normalize() first copies
the ctx accumulator to SBUF so the PSUM bank frees early (the next
q-tile's accumulators reuse it); ctxT_b and wo_b are mirrored into
partitions 64:127 so consecutive out_proj b-matmuls alternate PE row
groups and overlap.
"""

import numpy as np

D_MODEL = 768
NUM_HEADS = 12
D_K = 64
B = 2
S = 2048
N_CORES = 8
G = 4              # head groups (cores per batch)
GW = D_MODEL // G  # 192 features per group = 3 heads
HPG = 3            # heads per group
DC = D_MODEL // 128  # 6 d_model chunks
QT = 512           # q-tile width
NQT = S // QT      # 4
KC = S // 128      # 16 k chunks
ST = S // 128      # 16 seq tiles
BPK = 8            # packed bias columns
WA = BPK + 2 * DC * GW        # 2312: bias | wk | wq
WB = DC * GW + 2 * D_MODEL    # 2688: wv | wo_a | wo_b(mirrored)
XH = DC * 1024                # 6144: one x half (6 chunks x 1024 seq)
XVW = 4 * DC * 128            # 3072: one xv group (4 seq tiles)

_PROGRAM = None


def _build_program():
    from concourse import bacc, tile
    import concourse.mybir as mybir

    f16 = mybir.dt.float16
    f32 = mybir.dt.float32
    Exp = mybir.ActivationFunctionType.Exp
    mult = mybir.AluOpType.mult

    nc = bacc.Bacc("TRN2", target_bir_lowering=False, debug=False,
                   enable_asserts=False)

    xkP = nc.dram_tensor("xkP", [128, 2 * XH], f16, kind="ExternalInput")
    xqP = nc.dram_tensor("xqP", [128, 2 * XH], f16, kind="ExternalInput")
    xvR = nc.dram_tensor("xvR", [128, 4 * XVW], f16, kind="ExternalInput")
    wA = nc.dram_tensor("wA", [128, WA], f16, kind="ExternalInput")
    wB = nc.dram_tensor("wB", [128, WB], f16, kind="ExternalInput")
    out = nc.dram_tensor("out", [S, D_MODEL], f16, kind="ExternalOutput")

    with tile.TileContext(nc) as tc:
        with tc.tile_pool(name="const", bufs=1) as cp, \
             tc.tile_pool(name="expp", bufs=4) as ep, \
             tc.tile_pool(name="normp", bufs=2) as np_, \
             tc.tile_pool(name="outp", bufs=3) as op, \
             tc.tile_pool(name="ps_s", bufs=2, space="PSUM") as ps_s, \
             tc.tile_pool(name="ps_c", bufs=2, space="PSUM") as ps_c, \
             tc.tile_pool(name="ps_w", bufs=2, space="PSUM") as ps_w:

            # ---- DMA: wA, xk halves, xq half0, wB, xv-g0, xq half1,
            # xv g1-3 ----
            wa = cp.tile([128, WA], f16, name="wa")
            nc.sync.dma_start(out=wa[:], in_=wA[:])
            wk_sb = [wa[:, BPK + d * GW:BPK + (d + 1) * GW]
                     for d in range(DC)]
            wq_sb = [wa[:, BPK + DC * GW + d * GW:
                        BPK + DC * GW + (d + 1) * GW] for d in range(DC)]

            xk_sb = cp.tile([128, 2 * XH], f16, name="xk_sb")
            for h in range(2):
                nc.sync.dma_start(out=xk_sb[:, h * XH:(h + 1) * XH],
                                  in_=xkP[:, h * XH:(h + 1) * XH])
            xq_sb = cp.tile([128, 2 * XH], f16, name="xq_sb")
            nc.sync.dma_start(out=xq_sb[:, 0:XH], in_=xqP[:, 0:XH])

            wb = cp.tile([128, WB], f16, name="wb")
            nc.sync.dma_start(out=wb[:], in_=wB[:])
            wv_sb = [wb[:, d * GW:(d + 1) * GW] for d in range(DC)]
            wo_a = wb[:, DC * GW:DC * GW + D_MODEL]
            wo_bm = wb[:, DC * GW + D_MODEL:WB]

            xvg = [cp.tile([128, XVW], f16, name=f"xv{g}")
                   for g in range(4)]
            nc.sync.dma_start(out=xvg[0][:], in_=xvR[:, 0:XVW])
            nc.sync.dma_start(out=xq_sb[:, XH:2 * XH],
                              in_=xqP[:, XH:2 * XH])
            for g in range(1, 4):
                nc.sync.dma_start(out=xvg[g][:],
                                  in_=xvR[:, g * XVW:(g + 1) * XVW])

            def xk_v(d, c0, w):
                h, c1 = divmod(c0, 1024)
                return xk_sb[:, h * XH + d * 1024 + c1:
                             h * XH + d * 1024 + c1 + w]

            def xq_v(d, c0, w):
                h, c1 = divmod(c0, 1024)
                return xq_sb[:, h * XH + d * 1024 + c1:
                             h * XH + d * 1024 + c1 + w]

            # biases as f32 scalars (wa holds them as f16)
            bps = cp.tile([128, BPK], f32, name="bps")
            nc.vector.tensor_copy(out=bps[:], in_=wa[:, 0:BPK])
            bq_a, bq_b = bps[:, 0:1], bps[0:64, 1:2]
            bk_a, bk_b = bps[:, 2:3], bps[0:64, 3:4]
            bv_h = [bps[0:64, 4 + h:5 + h] for h in range(HPG)]

            # ---- PE warmup in the "w" pool; source is a zeroed tile ----
            junk = cp.tile([128, 256], f16, name="junk")
            nc.vector.memset(junk[:], 0.5)
            wupt = [ps_w.tile([128, QT], f32, name="W", tag="w")
                    for _ in range(2)]
            for i in range(10):
                nc.tensor.matmul(wupt[i % 2][:, 0:256], lhsT=junk[:, 0:128],
                                 rhs=junk[:], start=True, stop=True)
            # dummy exp pulls the ACT exp-table load into the DMA shadow
            escr = cp.tile([128, BPK], f16, name="escr")
            nc.scalar.activation(escr[:], bps[:], Exp, scale=0.001)

            # ---- pre-attention projections: k passA (both halves), q
            # passA columns 0:1024 ----
            qT_a = cp.tile([128, S], f16, name="qT_a")
            qT_b = cp.tile([128, S], f16, name="qT_b")
            kT_a = cp.tile([128, S], f16, name="kT_a")
            kT_b = cp.tile([128, S], f16, name="kT_b")

            for j2 in range(2):
                pj = ps_s.tile([128, 2 * QT], f32, name="S", tag="s")
                for d in range(DC):
                    for n in range(2):
                        nc.tensor.matmul(
                            pj[:, n * QT:(n + 1) * QT],
                            lhsT=wk_sb[d][:, 0:128],
                            rhs=xk_v(d, j2 * 1024 + n * QT, QT),
                            start=(d == 0), stop=(d == DC - 1))
                nc.vector.tensor_scalar_add(
                    kT_a[:, j2 * 1024:(j2 + 1) * 1024], pj[:], bk_a)
            pjQ = ps_s.tile([128, 2 * QT], f32, name="S", tag="s")
            for d in range(DC):
                for n in range(2):
                    nc.tensor.matmul(
                        pjQ[:, n * QT:(n + 1) * QT],
                        lhsT=wq_sb[d][:, 0:128], rhs=xq_v(d, n * QT, QT),
                        start=(d == 0), stop=(d == DC - 1))
            nc.vector.tensor_scalar_add(qT_a[:, 0:1024], pjQ[:], bq_a)

            # ---- deferred projection units: q passA cols 1024:2048, then
            # k/q passB (head 2). Each unit = 6 accumulating matmuls into a
            # "w" tile + a bias add. Injected into qt1-3's h01 slack. ----
            fillers = []

            def add_unit(xv_fn, wsel, w_lo, w_hi, b, dst, c0, last, mirror):
                state = {}
                rows = w_hi - w_lo

                def mk(d):
                    def emit():
                        if d == 0:
                            state["pj"] = ps_w.tile([128, QT], f32,
                                                    name="W", tag="w")
                        nc.tensor.matmul(
                            state["pj"][0:rows, :],
                            lhsT=wsel[d][:, w_lo:w_hi],
                            rhs=xv_fn(d, c0, QT),
                            start=(d == 0), stop=(d == DC - 1))
                        if d == DC - 1:
                            nc.vector.tensor_scalar_add(
                                dst[0:rows, c0:c0 + QT],
                                state["pj"][0:rows, :], b)
                            if last and mirror:
                                nc.sync.dma_start(out=dst[64:128, :],
                                                  in_=dst[0:64, :])
                    return emit
                for d in range(DC):
                    fillers.append(mk(d))

            for n in range(2):  # q passA cols 1024:2048
                add_unit(xq_v, wq_sb, 0, 128, bq_a, qT_a,
                         1024 + n * QT, False, False)
            for n4 in range(4):  # k passB
                add_unit(xk_v, wk_sb, 128, GW, bk_b, kT_b,
                         n4 * QT, n4 == 3, True)
            for n4 in range(4):  # q passB
                add_unit(xq_v, wq_sb, 128, GW, bq_b, qT_b,
                         n4 * QT, n4 == 3, True)

            # ---- v projection: just-in-time per seq tile inside qt0 ----
            v_sb = [None] * ST

            def v_proj(st):
                g4, s4 = st // 4, st % 4
                pv = ps_w.tile([128, QT], f32, name="W", tag="w")
                for d in range(DC):
                    c0 = (s4 * DC + d) * 128
                    nc.tensor.matmul(pv[:, 0:GW],
                                     lhsT=xvg[g4][:, c0:c0 + 128],
                                     rhs=wv_sb[d][:],
                                     start=(d == 0), stop=(d == DC - 1))
                vt = cp.tile([128, HPG, D_K + 1], f16, name=f"vsb{st}")
                nc.vector.tensor_copy(out=vt[:, :, 0:D_K],
                                      in_=pv[:, 0:GW].rearrange(
                                          "p (h w) -> p h w", h=HPG))
                nc.vector.memset(vt[:, :, D_K:D_K + 1], 1.0)
                v_sb[st] = vt

            # ---- attention state ----
            ctxT_a = [cp.tile([128, QT], f16, name=f"ctxTa{j}")
                      for j in range(NQT)]
            ctxT_b = [cp.tile([128, QT], f16, name=f"ctxTb{j}")
                      for j in range(NQT)]

            def normalize(C, h, qt):
                # Copy the accumulator (and denominator row) to SBUF first:
                # the PSUM bank frees after two quick copies instead of
                # after the whole chain, so the next q-tile's accumulators
                # never stall on it. h1's copy lands at partitions 64:128
                # so the multiply's SBUF operands share a start partition.
                base = 64 if h == 1 else 0
                ctx_dst = (ctxT_a[qt][0:64] if h == 0 else
                           ctxT_a[qt][64:128] if h == 1 else
                           ctxT_b[qt][0:64])
                den = np_.tile([1, QT], f32, name="den", tag="den")
                nc.vector.tensor_copy(out=den[:], in_=C[D_K:D_K + 1, :])
                Cc = np_.tile([128, QT], f32, name="Cc", tag="cc")
                nc.vector.tensor_copy(out=Cc[base:base + D_K, :],
                                      in_=C[0:D_K, :])
                r = np_.tile([1, QT], f32, name="r", tag="r")
                nc.vector.reciprocal_approx_fast(out=r[:], in_=den[:])
                bc = np_.tile([128, QT], f32, name="bc", tag="bc")
                nc.gpsimd.partition_broadcast(bc[:], r[:])
                nc.vector.tensor_tensor(out=ctx_dst[:],
                                        in0=Cc[base:base + D_K, :],
                                        in1=bc[base:base + D_K, :],
                                        op=mult)
                nc.vector.tensor_scalar_add(ctx_dst[:], ctx_dst[:], bv_h[h])
                if h == 2:
                    # mirror so out_proj b-matmuls can alternate row groups
                    nc.sync.dma_start(out=ctxT_b[qt][64:128, :],
                                      in_=ctxT_b[qt][0:64, :])

            osb = [None] * (ST // 4)

            def out_unit(qt, u, po=None, tail=False):
                # one (st, ns) unit of q-tile qt's output projection
                st, ns_i = u // 2, u % 2
                ws = slice(st * 128, (st + 1) * 128)
                ns = slice(ns_i * 384, (ns_i + 1) * 384)
                if ns_i == 0:
                    osb[st] = op.tile([128, D_MODEL], f16, name="osb")
                if po is None:
                    po = ps_w.tile([128, QT], f32, name="W", tag="w")[:, 0:384]
                nc.tensor.matmul(po[:], lhsT=ctxT_a[qt][:, ws],
                                 rhs=wo_a[:, ns], start=True, stop=False)
                rb = slice(0, 64) if u % 2 == 0 else slice(64, 128)
                nc.tensor.matmul(po[:], lhsT=ctxT_b[qt][rb, ws],
                                 rhs=wo_bm[rb, ns], start=False, stop=True)
                if tail and u % 2 == 1:
                    nc.scalar.copy(osb[st][:, ns], po[:])
                else:
                    nc.vector.tensor_copy(out=osb[st][:, ns], in_=po[:])
                if ns_i == 1:
                    r0 = qt * QT + st * 128
                    nc.sync.dma_start(out=out[r0:r0 + 128, :], in_=osb[st][:])

            def attn_hp01(qt):
                # heads 0+1: row-group-paired score matmuls into one
                # [128,1024] PSUM tile, exp as a single wide op.
                qs = slice(qt * QT, (qt + 1) * QT)
                Cs = {h: ps_c.tile([128, QT], f32, name="C", tag="c")
                      for h in (0, 1)}
                S2s = [None] * KC

                def scores(kc):
                    ks = slice(kc * 128, (kc + 1) * 128)
                    S2 = ps_s.tile([128, 2 * QT], f32, name="S", tag="s")
                    nc.tensor.matmul(S2[:, 0:QT], lhsT=kT_a[0:64, ks],
                                     rhs=qT_a[0:64, qs])
                    nc.tensor.matmul(S2[:, QT:2 * QT], lhsT=kT_a[64:128, ks],
                                     rhs=qT_a[64:128, qs])
                    S2s[kc] = S2

                scores(0)
                for kc in range(KC):
                    e2 = ep.tile([128, 2 * QT], f16, name="expT")
                    nc.scalar.activation(e2[:], S2s[kc][:], Exp, scale=0.125)
                    if kc + 1 < KC:
                        scores(kc + 1)
                    if qt == 0:
                        v_proj(kc)
                    else:
                        rate = 2 if (qt == 1 and kc < 12) else 1
                        for _ in range(rate):
                            if fillers:
                                fillers.pop(0)()
                    for h in (0, 1):
                        nc.tensor.matmul(Cs[h][0:D_K + 1, :],
                                         lhsT=v_sb[kc][:, h, :],
                                         rhs=e2[:, h * QT:(h + 1) * QT],
                                         start=(kc == 0), stop=(kc == KC - 1))
                for h in (0, 1):
                    normalize(Cs[h], h, qt)

            def attn_h2(qt, oqt):
                # head 2: one [128,1024] scores tile covers two k-chunks via
                # the mirrored b-half; out_proj units for q-tile oqt are
                # injected one per iteration.
                qs = slice(qt * QT, (qt + 1) * QT)
                C2 = ps_c.tile([128, QT], f32, name="C", tag="c")
                S2s = [None] * (KC // 2)

                def scores2(kc2):
                    S2 = ps_s.tile([128, 2 * QT], f32, name="S", tag="s")
                    for i in (0, 1):
                        kc = 2 * kc2 + i
                        rg = slice(64 * i, 64 * i + 64)
                        nc.tensor.matmul(
                            S2[:, i * QT:(i + 1) * QT],
                            lhsT=kT_b[rg, kc * 128:(kc + 1) * 128],
                            rhs=qT_b[rg, qs])
                    S2s[kc2] = S2

                scores2(0)
                for kc2 in range(KC // 2):
                    e2 = ep.tile([128, 2 * QT], f16, name="expT")
                    nc.scalar.activation(e2[:], S2s[kc2][:], Exp, scale=0.125)
                    if kc2 + 1 < KC // 2:
                        scores2(kc2 + 1)
                    for i in (0, 1):
                        kc = 2 * kc2 + i
                        nc.tensor.matmul(C2[0:D_K + 1, :],
                                         lhsT=v_sb[kc][:, 2, :],
                                         rhs=e2[:, i * QT:(i + 1) * QT],
                                         start=(kc == 0), stop=(kc == KC - 1))
                    if oqt is not None:
                        out_unit(oqt, kc2)
                normalize(C2, 2, qt)

            for qt in range(NQT):
                attn_hp01(qt)
            for qt in range(NQT):
                attn_h2(qt, qt - 1 if qt > 0 else None)
            # tail: qt3's out_proj. Keep the PE warm through the normalize
            # latency, then run 4 units from borrowed score banks + 4 from
            # the "w" pool so matmuls never wait on copies.
            ts = [ps_s.tile([128, 2 * QT], f32, name="S", tag="s")
                  for _ in range(2)]
            for i in range(6):
                nc.tensor.matmul(ts[i % 2][:, 0:256], lhsT=junk[:, 0:128],
                                 rhs=junk[:], start=True, stop=True)
            slots = [ts[0][:, 0:384], ts[0][:, QT:QT + 384],
                     ts[1][:, 0:384], ts[1][:, QT:QT + 384],
                     None, None, None, None]
            for u in range(8):
                out_unit(NQT - 1, u, po=slots[u], tail=True)

    nc.compile()
    return nc


def _get_program():
    global _PROGRAM
    if _PROGRAM is None:
        _PROGRAM = _build_program()
    return _PROGRAM


def make_in_maps(query, key, value, Wq, bq, Wk, bk, Wv, bv, Wo, bo):
    """Build the 8 per-core input maps (host-side shard + pack + cast)."""
    q32 = np.asarray(query, np.float32)
    k32 = np.asarray(key, np.float32)
    v32 = np.asarray(value, np.float32)

    def pack_x(xT):
        # [768, 2048] -> [128, 2*6144]: halves x chunks x 1024, so each
        # SBUF partition line is one contiguous 12KB DMA line per half
        return np.ascontiguousarray(
            xT.reshape(DC, 128, 2, 1024).transpose(2, 1, 0, 3)
        ).reshape(2, 128, XH).transpose(1, 0, 2).reshape(128, 2 * XH)

    def pack_v(xT):
        # [768, 2048] -> [128, 4*3072]: seq-tile-quad groups
        return np.ascontiguousarray(
            xT.reshape(DC, 128, ST, 128).transpose(1, 2, 0, 3)
        ).reshape(128, ST * DC * 128)

    xP = {}
    for b in range(B):
        xP[b] = (pack_x(q32[b].T.astype(np.float16)),
                 pack_x(k32[b].T.astype(np.float16)),
                 pack_v(v32[b].T.astype(np.float16)))
    Wq = np.asarray(Wq, np.float32)
    Wk = np.asarray(Wk, np.float32)
    Wv = np.asarray(Wv, np.float32)
    Wo = np.asarray(Wo, np.float32)
    bq = np.asarray(bq, np.float32)
    bk = np.asarray(bk, np.float32)
    bv = np.asarray(bv, np.float32)
    in_maps = []
    for c in range(N_CORES):
        b, g = divmod(c, G)
        fs = slice(g * GW, (g + 1) * GW)
        xq, xk, xv = xP[b]
        wa = np.zeros((128, WA), np.float16)
        wa[:, 0] = bq[fs][0:128]
        wa[0:64, 1] = bq[fs][128:GW]
        wa[:, 2] = bk[fs][0:128]
        wa[0:64, 3] = bk[fs][128:GW]
        for h in range(HPG):
            wa[0:64, 4 + h] = bv[fs][h * 64:(h + 1) * 64]
        for i, W in enumerate((Wk, Wq)):
            Ws = W[:, fs]
            for d in range(DC):
                c0 = BPK + (i * DC + d) * GW
                wa[:, c0:c0 + GW] = Ws[d * 128:(d + 1) * 128, :].astype(
                    np.float16)
        wbp = np.zeros((128, WB), np.float16)
        Ws = Wv[:, fs]
        for d in range(DC):
            wbp[:, d * GW:(d + 1) * GW] = \
                Ws[d * 128:(d + 1) * 128, :].astype(np.float16)
        Wos = Wo[fs, :]
        wbp[:, DC * GW:DC * GW + D_MODEL] = Wos[0:128, :].astype(np.float16)
        wob = Wos[128:GW, :].astype(np.float16)
        wbp[0:64, DC * GW + D_MODEL:WB] = wob
        wbp[64:128, DC * GW + D_MODEL:WB] = wob
        in_maps.append({
            "xqP": xq, "xkP": xk, "xvR": xv,
            "wA": wa, "wB": wbp,
        })
    return in_maps


def combine_outputs(results, bo):
    """Sum the per-core partial outputs into the full [B, S, D] output."""
    bo = np.asarray(bo, np.float32)
    out = np.zeros((B, S, D_MODEL), np.float32)
    for c in range(N_CORES):
        b = c // G
        out[b] += np.asarray(results[c]["out"], np.float32)
    out += bo[None, None, :]
    return out


def kernel(**inputs):
    from concourse.bass_utils import run_bass_kernel_spmd

    nc = _get_program()
    in_maps = make_in_maps(**inputs)
    res = run_bass_kernel_spmd(nc, in_maps, list(range(N_CORES)))
    return combine_outputs(res.results, inputs["bo"])


# revision 18
# speedup vs baseline: 1.1152x; 1.0307x over previous
"""Multi-head attention (B=2, S=2048, D=768, H=12) on 8 trn2 NeuronCores.

Sharding: batch x head-group data/tensor parallel. Core c = b*4+g handles
batch b and heads [3g, 3g+3) (a 192-wide slice of the QKV projections and
the matching 192-row slice of Wo). Each core emits a partial [2048, 768]
fp16 output; the host sums the 4 head-group partials per batch and adds bo.

Device schedule. The kernel is dual-roofline (~100us PE streaming, ~97us
ACT exp). The DMA engines sustain ~330GB/s only with large contiguous
lines (they are descriptor-bound at ~10ns/partition-line), so inputs are
host-packed:
  wA  [128, 2312]   biases(f16) | wk | wq            (4.6KB lines)
  xqP [128, 12288]  column halves x chunks x 1024    (4KB-line pair xfers)
  xkP [128, 12288]  d-major chunks x 2048            (8KB-line pair xfers)
  wB  [128, 2688]   wv | wo_a | wo_b mirrored        (5.4KB lines)
  xvR [128, 12288]  seq-tile-quad groups             (6KB lines)
DMA order: wA, xq-half0 (3 chunk-pair transfers), xk (3 pair transfers),
wB, xv groups, xq-half1. Projections consume each pair as it lands; the
first exp fires ~23us in. Warmup junk matmuls bridge the DMA wait so the
PE HAM clock gate stays open.

Only k passA and q passA columns 0:1024 run before attention. The rest
(q passA cols 1024:2048, k/q passB) is deferred into attention PE slack
as six-matmul units cycling through the 2-bank "w" PSUM pool; the v
projection runs just-in-time per seq tile inside qt0.

The attention itself is ONE flat software-pipelined stream over phases
h01(qt0..3) then h2(qt0..3): at every step the NEXT step's score matmuls
are emitted before this step's ctx matmuls — across phase boundaries too
— so the scalar engine's monotonic semaphore wait for exp[i+1] never
covers ctx[i] and the exp stream never drains at a boundary. h2 phases
carry the previous q-tile's output projection, one (st, ns) unit per
iteration; out_proj(qt3) is the tail (borrowing idle score banks so four
units are in flight, copies alternating Scalar/Vector). normalize()
copies the accumulator to SBUF immediately so the PSUM bank frees early;
ctxT_b and wo_b are mirrored into partitions 64:127 so out_proj b-matmuls
alternate PE row groups.
"""

import numpy as np

D_MODEL = 768
NUM_HEADS = 12
D_K = 64
B = 2
S = 2048
N_CORES = 8
G = 4              # head groups (cores per batch)
GW = D_MODEL // G  # 192 features per group = 3 heads
HPG = 3            # heads per group
DC = D_MODEL // 128  # 6 d_model chunks
QT = 512           # q-tile width
NQT = S // QT      # 4
KC = S // 128      # 16 k chunks
ST = S // 128      # 16 seq tiles
BPK = 8            # packed bias columns
WA = BPK + 2 * DC * GW        # 2312: bias | wk | wq
WB = DC * GW + 2 * D_MODEL    # 2688: wv | wo_a | wo_b(mirrored)
XH = DC * 1024                # 6144: one xq half (6 chunks x 1024 seq)
XVW = 4 * DC * 128            # 3072: one xv group (4 seq tiles)

_PROGRAM = None


def _build_program():
    from concourse import bacc, tile
    import concourse.mybir as mybir

    f16 = mybir.dt.float16
    f32 = mybir.dt.float32
    Exp = mybir.ActivationFunctionType.Exp
    mult = mybir.AluOpType.mult

    nc = bacc.Bacc("TRN2", target_bir_lowering=False, debug=False,
                   enable_asserts=False)

    xkP = nc.dram_tensor("xkP", [128, DC * S], f16, kind="ExternalInput")
    xqP = nc.dram_tensor("xqP", [128, 2 * XH], f16, kind="ExternalInput")
    xvR = nc.dram_tensor("xvR", [128, 4 * XVW], f16, kind="ExternalInput")
    wA = nc.dram_tensor("wA", [128, WA], f16, kind="ExternalInput")
    wB = nc.dram_tensor("wB", [128, WB], f16, kind="ExternalInput")
    out = nc.dram_tensor("out", [S, D_MODEL], f16, kind="ExternalOutput")

    with tile.TileContext(nc) as tc:
        with tc.tile_pool(name="const", bufs=1) as cp, \
             tc.tile_pool(name="expp", bufs=4) as ep, \
             tc.tile_pool(name="normp", bufs=2) as np_, \
             tc.tile_pool(name="outp", bufs=3) as op, \
             tc.tile_pool(name="ps_s", bufs=2, space="PSUM") as ps_s, \
             tc.tile_pool(name="ps_c", bufs=2, space="PSUM") as ps_c, \
             tc.tile_pool(name="ps_w", bufs=2, space="PSUM") as ps_w:

            # ---- DMA ----
            wa = cp.tile([128, WA], f16, name="wa")
            nc.sync.dma_start(out=wa[:], in_=wA[:])
            wk_sb = [wa[:, BPK + d * GW:BPK + (d + 1) * GW]
                     for d in range(DC)]
            wq_sb = [wa[:, BPK + DC * GW + d * GW:
                        BPK + DC * GW + (d + 1) * GW] for d in range(DC)]

            xq_sb = cp.tile([128, 2 * XH], f16, name="xq_sb")
            for p3 in range(3):
                nc.sync.dma_start(
                    out=xq_sb[:, p3 * 2048:(p3 + 1) * 2048],
                    in_=xqP[:, p3 * 2048:(p3 + 1) * 2048])
            xk_sb = cp.tile([128, DC * S], f16, name="xk_sb")
            for p3 in range(3):
                nc.sync.dma_start(
                    out=xk_sb[:, p3 * 4096:(p3 + 1) * 4096],
                    in_=xkP[:, p3 * 4096:(p3 + 1) * 4096])

            wb = cp.tile([128, WB], f16, name="wb")
            nc.sync.dma_start(out=wb[:], in_=wB[:])
            wv_sb = [wb[:, d * GW:(d + 1) * GW] for d in range(DC)]
            wo_a = wb[:, DC * GW:DC * GW + D_MODEL]
            wo_bm = wb[:, DC * GW + D_MODEL:WB]

            xvg = [cp.tile([128, XVW], f16, name=f"xv{g}")
                   for g in range(4)]
            for g in range(4):
                nc.sync.dma_start(out=xvg[g][:],
                                  in_=xvR[:, g * XVW:(g + 1) * XVW])
            nc.sync.dma_start(out=xq_sb[:, XH:2 * XH],
                              in_=xqP[:, XH:2 * XH])

            def xk_v(d, c0, w):
                return xk_sb[:, d * 2048 + c0:d * 2048 + c0 + w]

            def xq_v(d, c0, w):
                h, c1 = divmod(c0, 1024)
                return xq_sb[:, h * XH + d * 1024 + c1:
                             h * XH + d * 1024 + c1 + w]

            # biases as f32 scalars (wa holds them as f16)
            bps = cp.tile([128, BPK], f32, name="bps")
            nc.vector.tensor_copy(out=bps[:], in_=wa[:, 0:BPK])
            bq_a, bq_b = bps[:, 0:1], bps[0:64, 1:2]
            bk_a, bk_b = bps[:, 2:3], bps[0:64, 3:4]
            bv_h = [bps[0:64, 4 + h:5 + h] for h in range(HPG)]

            # ---- PE warmup bridging the DMA wait ----
            junk = cp.tile([128, QT], f16, name="junk")
            nc.vector.memset(junk[:], 0.5)
            wupt = [ps_w.tile([128, QT], f32, name="W", tag="w")
                    for _ in range(2)]
            for i in range(24):
                nc.tensor.matmul(wupt[i % 2][:], lhsT=junk[:, 0:128],
                                 rhs=junk[:], start=True, stop=True)
            # dummy exp pulls the ACT exp-table load into the DMA shadow
            escr = cp.tile([128, BPK], f16, name="escr")
            nc.scalar.activation(escr[:], bps[:], Exp, scale=0.001)

            # ---- pre-attention projections, consuming chunk pairs as
            # they land: q passA cols 0:1024, then k passA (all cols) ----
            qT_a = cp.tile([128, S], f16, name="qT_a")
            qT_b = cp.tile([128, S], f16, name="qT_b")
            kT_a = cp.tile([128, S], f16, name="kT_a")
            kT_b = cp.tile([128, S], f16, name="kT_b")

            pjQ = ps_s.tile([128, 2 * QT], f32, name="S", tag="s")
            for d in range(DC):
                for n in range(2):
                    nc.tensor.matmul(
                        pjQ[:, n * QT:(n + 1) * QT],
                        lhsT=wq_sb[d][:, 0:128], rhs=xq_v(d, n * QT, QT),
                        start=(d == 0), stop=(d == DC - 1))
            for n in range(2):
                nc.vector.tensor_scalar_add(
                    qT_a[:, n * QT:(n + 1) * QT],
                    pjQ[:, n * QT:(n + 1) * QT], bq_a)

            pjK = [ps_s.tile([128, 2 * QT], f32, name="S", tag="s")
                   for _ in range(2)]
            for d in range(DC):
                for j2 in range(2):
                    for n in range(2):
                        nc.tensor.matmul(
                            pjK[j2][:, n * QT:(n + 1) * QT],
                            lhsT=wk_sb[d][:, 0:128],
                            rhs=xk_v(d, j2 * 1024 + n * QT, QT),
                            start=(d == 0), stop=(d == DC - 1))
            for j2 in range(2):
                nc.vector.tensor_scalar_add(
                    kT_a[:, j2 * 1024:(j2 + 1) * 1024], pjK[j2][:], bk_a)

            # ---- deferred projection units ----
            fillers = []

            def add_unit(xv_fn, wsel, w_lo, w_hi, b, dst, c0, last, mirror):
                state = {}
                rows = w_hi - w_lo

                def mk(d):
                    def emit():
                        if d == 0:
                            state["pj"] = ps_w.tile([128, QT], f32,
                                                    name="W", tag="w")
                        nc.tensor.matmul(
                            state["pj"][0:rows, :],
                            lhsT=wsel[d][:, w_lo:w_hi],
                            rhs=xv_fn(d, c0, QT),
                            start=(d == 0), stop=(d == DC - 1))
                        if d == DC - 1:
                            nc.vector.tensor_scalar_add(
                                dst[0:rows, c0:c0 + QT],
                                state["pj"][0:rows, :], b)
                            if mirror:
                                nc.sync.dma_start(
                                    out=dst[64:128, c0:c0 + QT],
                                    in_=dst[0:64, c0:c0 + QT])
                    return emit
                for d in range(DC):
                    fillers.append(mk(d))

            for n in range(2):  # q passA cols 1024:2048
                add_unit(xq_v, wq_sb, 0, 128, bq_a, qT_a,
                         1024 + n * QT, False, False)
            for n4 in range(4):  # k passB
                add_unit(xk_v, wk_sb, 128, GW, bk_b, kT_b,
                         n4 * QT, n4 == 3, True)
            for n4 in range(4):  # q passB
                add_unit(xq_v, wq_sb, 128, GW, bq_b, qT_b,
                         n4 * QT, n4 == 3, True)

            # ---- v projection: just-in-time per seq tile inside qt0 ----
            v_sb = [None] * ST

            def v_proj(st):
                g4, s4 = st // 4, st % 4
                pv = ps_w.tile([128, QT], f32, name="W", tag="w")
                for d in range(DC):
                    c0 = (s4 * DC + d) * 128
                    nc.tensor.matmul(pv[:, 0:GW],
                                     lhsT=xvg[g4][:, c0:c0 + 128],
                                     rhs=wv_sb[d][:],
                                     start=(d == 0), stop=(d == DC - 1))
                vt = cp.tile([128, HPG, D_K + 1], f16, name=f"vsb{st}")
                nc.vector.tensor_copy(out=vt[:, :, 0:D_K],
                                      in_=pv[:, 0:GW].rearrange(
                                          "p (h w) -> p h w", h=HPG))
                nc.vector.memset(vt[:, :, D_K:D_K + 1], 1.0)
                v_sb[st] = vt

            # ---- attention state ----
            ctxT_a = [cp.tile([128, QT], f16, name=f"ctxTa{j}")
                      for j in range(NQT)]
            ctxT_b = [cp.tile([128, QT], f16, name=f"ctxTb{j}")
                      for j in range(NQT)]

            def normalize(C, h, qt):
                # Copy the accumulator (and denominator row) to SBUF first
                # so the PSUM bank frees early. h1's copy lands at
                # partitions 64:128 so the multiply's SBUF operands share a
                # start partition.
                base = 64 if h == 1 else 0
                ctx_dst = (ctxT_a[qt][0:64] if h == 0 else
                           ctxT_a[qt][64:128] if h == 1 else
                           ctxT_b[qt][0:64])
                den = np_.tile([1, QT], f32, name="den", tag="den")
                nc.vector.tensor_copy(out=den[:], in_=C[D_K:D_K + 1, :])
                Cc = np_.tile([128, QT], f32, name="Cc", tag="cc")
                nc.vector.tensor_copy(out=Cc[base:base + D_K, :],
                                      in_=C[0:D_K, :])
                r = np_.tile([1, QT], f32, name="r", tag="r")
                nc.vector.reciprocal_approx_fast(out=r[:], in_=den[:])
                bc = np_.tile([128, QT], f32, name="bc", tag="bc")
                nc.gpsimd.partition_broadcast(bc[:], r[:])
                nc.vector.tensor_tensor(out=ctx_dst[:],
                                        in0=Cc[base:base + D_K, :],
                                        in1=bc[base:base + D_K, :],
                                        op=mult)
                nc.vector.tensor_scalar_add(ctx_dst[:], ctx_dst[:], bv_h[h])
                if h == 2:
                    # mirror so out_proj b-matmuls can alternate row groups
                    nc.sync.dma_start(out=ctxT_b[qt][64:128, :],
                                      in_=ctxT_b[qt][0:64, :])

            osb = [None] * (ST // 4)

            def out_unit(qt, u, po=None, tail=False):
                # one (st, ns) unit of q-tile qt's output projection
                st, ns_i = u // 2, u % 2
                ws = slice(st * 128, (st + 1) * 128)
                ns = slice(ns_i * 384, (ns_i + 1) * 384)
                if ns_i == 0:
                    osb[st] = op.tile([128, D_MODEL], f16, name="osb")
                if po is None:
                    po = ps_w.tile([128, QT], f32, name="W",
                                   tag="w")[:, 0:384]
                nc.tensor.matmul(po[:], lhsT=ctxT_a[qt][:, ws],
                                 rhs=wo_a[:, ns], start=True, stop=False)
                rb = slice(0, 64) if u % 2 == 0 else slice(64, 128)
                nc.tensor.matmul(po[:], lhsT=ctxT_b[qt][rb, ws],
                                 rhs=wo_bm[rb, ns], start=False, stop=True)
                if tail and u % 2 == 1:
                    nc.scalar.copy(osb[st][:, ns], po[:])
                else:
                    nc.vector.tensor_copy(out=osb[st][:, ns], in_=po[:])
                if ns_i == 1:
                    r0 = qt * QT + st * 128
                    nc.sync.dma_start(out=out[r0:r0 + 128, :], in_=osb[st][:])

            # ---- the flat attention stream ----
            # phase descriptors: ("h01", qt) x4 then ("h2", qt) x4
            steps = []
            for qt in range(NQT):
                steps += [("h01", qt, kc) for kc in range(KC)]
            for qt in range(NQT):
                steps += [("h2", qt, kc2) for kc2 in range(KC // 2)]

            Cs = {}   # (kind, qt) -> accumulator tile(s)
            S2q = [None] * len(steps)

            def emit_scores(i):
                kind, qt, kc = steps[i]
                S2 = ps_s.tile([128, 2 * QT], f32, name="S", tag="s")
                qs = slice(qt * QT, (qt + 1) * QT)
                if kind == "h01":
                    ks = slice(kc * 128, (kc + 1) * 128)
                    nc.tensor.matmul(S2[:, 0:QT], lhsT=kT_a[0:64, ks],
                                     rhs=qT_a[0:64, qs])
                    nc.tensor.matmul(S2[:, QT:2 * QT],
                                     lhsT=kT_a[64:128, ks],
                                     rhs=qT_a[64:128, qs])
                else:
                    for ii in (0, 1):
                        kcc = 2 * kc + ii
                        rg = slice(64 * ii, 64 * ii + 64)
                        nc.tensor.matmul(
                            S2[:, ii * QT:(ii + 1) * QT],
                            lhsT=kT_b[rg, kcc * 128:(kcc + 1) * 128],
                            rhs=qT_b[rg, qs])
                S2q[i] = S2

            emit_scores(0)
            for i, (kind, qt, kc) in enumerate(steps):
                e2 = ep.tile([128, 2 * QT], f16, name="expT")
                nc.scalar.activation(e2[:], S2q[i][:], Exp, scale=0.125)
                S2q[i] = None
                if i + 1 < len(steps):
                    emit_scores(i + 1)
                # PE-slack extras
                if kind == "h01":
                    if qt == 0:
                        v_proj(kc)
                    else:
                        rate = 2 if (qt == 1 and kc < 4) else 1
                        for _ in range(rate):
                            if fillers:
                                fillers.pop(0)()
                else:
                    if qt > 0:
                        out_unit(qt - 1, kc)
                    elif fillers:
                        fillers.pop(0)()
                # ctx
                if kind == "h01":
                    if kc == 0:
                        Cs[qt] = {h: ps_c.tile([128, QT], f32, name="C",
                                               tag="c") for h in (0, 1)}
                    for h in (0, 1):
                        nc.tensor.matmul(Cs[qt][h][0:D_K + 1, :],
                                         lhsT=v_sb[kc][:, h, :],
                                         rhs=e2[:, h * QT:(h + 1) * QT],
                                         start=(kc == 0), stop=(kc == KC - 1))
                    if kc == KC - 1:
                        for h in (0, 1):
                            normalize(Cs[qt][h], h, qt)
                else:
                    if kc == 0:
                        Cs[("h2", qt)] = ps_c.tile([128, QT], f32,
                                                   name="C", tag="c")
                    C2 = Cs[("h2", qt)]
                    for ii in (0, 1):
                        kcc = 2 * kc + ii
                        nc.tensor.matmul(C2[0:D_K + 1, :],
                                         lhsT=v_sb[kcc][:, 2, :],
                                         rhs=e2[:, ii * QT:(ii + 1) * QT],
                                         start=(kcc == 0),
                                         stop=(kcc == KC - 1))
                    if kc == KC // 2 - 1:
                        normalize(C2, 2, qt)

            # ---- tail: qt3's out_proj. Keep the PE warm through the
            # normalize latency; borrow idle score banks so four units are
            # in flight. ----
            ts = [ps_s.tile([128, 2 * QT], f32, name="S", tag="s")
                  for _ in range(2)]
            for i in range(6):
                nc.tensor.matmul(ts[i % 2][:, 0:256], lhsT=junk[:, 0:128],
                                 rhs=junk[:, 0:256], start=True, stop=True)
            slots = [ts[0][:, 0:384], ts[0][:, QT:QT + 384],
                     ts[1][:, 0:384], ts[1][:, QT:QT + 384],
                     None, None, None, None]
            for u in range(8):
                out_unit(NQT - 1, u, po=slots[u], tail=True)

    nc.compile()
    return nc


def _get_program():
    global _PROGRAM
    if _PROGRAM is None:
        _PROGRAM = _build_program()
    return _PROGRAM


def make_in_maps(query, key, value, Wq, bq, Wk, bk, Wv, bv, Wo, bo):
    """Build the 8 per-core input maps (host-side shard + pack + cast)."""
    q32 = np.asarray(query, np.float32)
    k32 = np.asarray(key, np.float32)
    v32 = np.asarray(value, np.float32)

    def pack_q(xT):
        # [768, 2048] -> [128, 2*6144]: halves x chunks x 1024
        return np.ascontiguousarray(
            xT.reshape(DC, 128, 2, 1024).transpose(2, 1, 0, 3)
        ).reshape(2, 128, XH).transpose(1, 0, 2).reshape(128, 2 * XH)

    def pack_k(xT):
        # [768, 2048] -> [128, 6*2048]: d-major full-width chunks
        return np.ascontiguousarray(
            xT.reshape(DC, 128, S).transpose(1, 0, 2)).reshape(128, DC * S)

    def pack_v(xT):
        # [768, 2048] -> [128, 16*768]: seq-tile-major
        return np.ascontiguousarray(
            xT.reshape(DC, 128, ST, 128).transpose(1, 2, 0, 3)
        ).reshape(128, ST * DC * 128)

    xP = {}
    for b in range(B):
        xP[b] = (pack_q(q32[b].T.astype(np.float16)),
                 pack_k(k32[b].T.astype(np.float16)),
                 pack_v(v32[b].T.astype(np.float16)))
    Wq = np.asarray(Wq, np.float32)
    Wk = np.asarray(Wk, np.float32)
    Wv = np.asarray(Wv, np.float32)
    Wo = np.asarray(Wo, np.float32)
    bq = np.asarray(bq, np.float32)
    bk = np.asarray(bk, np.float32)
    bv = np.asarray(bv, np.float32)
    in_maps = []
    for c in range(N_CORES):
        b, g = divmod(c, G)
        fs = slice(g * GW, (g + 1) * GW)
        xq, xk, xv = xP[b]
        wa = np.zeros((128, WA), np.float16)
        wa[:, 0] = bq[fs][0:128]
        wa[0:64, 1] = bq[fs][128:GW]
        wa[:, 2] = bk[fs][0:128]
        wa[0:64, 3] = bk[fs][128:GW]
        for h in range(HPG):
            wa[0:64, 4 + h] = bv[fs][h * 64:(h + 1) * 64]
        for i, W in enumerate((Wk, Wq)):
            Ws = W[:, fs]
            for d in range(DC):
                c0 = BPK + (i * DC + d) * GW
                wa[:, c0:c0 + GW] = Ws[d * 128:(d + 1) * 128, :].astype(
                    np.float16)
        wbp = np.zeros((128, WB), np.float16)
        Ws = Wv[:, fs]
        for d in range(DC):
            wbp[:, d * GW:(d + 1) * GW] = \
                Ws[d * 128:(d + 1) * 128, :].astype(np.float16)
        Wos = Wo[fs, :]
        wbp[:, DC * GW:DC * GW + D_MODEL] = Wos[0:128, :].astype(np.float16)
        wob = Wos[128:GW, :].astype(np.float16)
        wbp[0:64, DC * GW + D_MODEL:WB] = wob
        wbp[64:128, DC * GW + D_MODEL:WB] = wob
        in_maps.append({
            "xqP": xq, "xkP": xk, "xvR": xv,
            "wA": wa, "wB": wbp,
        })
    return in_maps


def combine_outputs(results, bo):
    """Sum the per-core partial outputs into the full [B, S, D] output."""
    bo = np.asarray(bo, np.float32)
    out = np.zeros((B, S, D_MODEL), np.float32)
    for c in range(N_CORES):
        b = c // G
        out[b] += np.asarray(results[c]["out"], np.float32)
    out += bo[None, None, :]
    return out


def kernel(**inputs):
    from concourse.bass_utils import run_bass_kernel_spmd

    nc = _get_program()
    in_maps = make_in_maps(**inputs)
    res = run_bass_kernel_spmd(nc, in_maps, list(range(N_CORES)))
    return combine_outputs(res.results, inputs["bo"])


# revision 24
# speedup vs baseline: 1.1317x; 1.0148x over previous
"""Multi-head attention (B=2, S=2048, D=768, H=12) on 8 trn2 NeuronCores.

Sharding: batch x head-group data/tensor parallel. Core c = b*4+g handles
batch b and heads [3g, 3g+3) (a 192-wide slice of the QKV projections and
the matching 192-row slice of Wo). Each core emits a partial [2048, 768]
fp16 output; the host sums the 4 head-group partials per batch and adds bo.

Device schedule. The kernel is dual-roofline (~100us PE streaming, ~97us
ACT exp). The DMA engines sustain ~330GB/s only with large contiguous
lines (they are descriptor-bound at ~10ns/partition-line), so inputs are
host-packed:
  wA  [128, 2312]   biases(f16) | wk | wq            (4.6KB lines)
  xqP [128, 12288]  column halves x chunks x 1024    (4KB-line pair xfers)
  xkP [128, 12288]  d-major chunks x 2048            (8KB-line pair xfers)
  wB  [128, 2688]   wv | wo_a | wo_b mirrored        (5.4KB lines)
  xvR [128, 12288]  seq-tile-quad groups             (6KB lines)
DMA order: wA, xq-half0 (3 chunk-pair transfers), xk (3 pair transfers),
wB, xv groups, xq-half1. Projections consume each pair as it lands; the
first exp fires ~23us in. Warmup junk matmuls bridge the DMA wait so the
PE HAM clock gate stays open.

Only k passA and q passA columns 0:1024 run before attention. The rest
(q passA cols 1024:2048, k/q passB) is deferred into attention PE slack
as six-matmul units cycling through the 2-bank "w" PSUM pool; the v
projection runs just-in-time per seq tile inside qt0.

The attention itself is ONE flat software-pipelined stream over phases
h01(qt0..3) then h2(qt0..3): at every step the NEXT step's score matmuls
are emitted before this step's ctx matmuls — across phase boundaries too
— so the scalar engine's monotonic semaphore wait for exp[i+1] never
covers ctx[i] and the exp stream never drains at a boundary. h2 phases
carry the previous q-tile's output projection, one (st, ns) unit per
iteration; out_proj(qt3) is the tail (borrowing idle score banks so four
units are in flight, copies alternating Scalar/Vector). normalize()
copies the accumulator to SBUF immediately so the PSUM bank frees early;
ctxT_b and wo_b are mirrored into partitions 64:127 so out_proj b-matmuls
alternate PE row groups.
"""

import numpy as np

D_MODEL = 768
NUM_HEADS = 12
D_K = 64
B = 2
S = 2048
N_CORES = 8
G = 4              # head groups (cores per batch)
GW = D_MODEL // G  # 192 features per group = 3 heads
HPG = 3            # heads per group
DC = D_MODEL // 128  # 6 d_model chunks
QT = 512           # q-tile width
NQT = S // QT      # 4
KC = S // 128      # 16 k chunks
ST = S // 128      # 16 seq tiles
BPK = 8            # packed bias columns
WA = BPK + 2 * DC * GW        # 2312: bias | wk | wq
WB = DC * GW + 2 * D_MODEL    # 2688: wv | wo_a | wo_b(mirrored)
XH = DC * 1024                # 6144: one xq half (6 chunks x 1024 seq)
XVW = 4 * DC * 128            # 3072: one xv group (4 seq tiles)

_PROGRAM = None


def _build_program():
    from concourse import bacc, tile
    import concourse.mybir as mybir

    f16 = mybir.dt.float16
    f32 = mybir.dt.float32
    Exp = mybir.ActivationFunctionType.Exp
    mult = mybir.AluOpType.mult

    nc = bacc.Bacc("TRN2", target_bir_lowering=False, debug=False,
                   enable_asserts=False)

    xkP = nc.dram_tensor("xkP", [128, DC * S], f16, kind="ExternalInput")
    xqP = nc.dram_tensor("xqP", [128, 2 * XH], f16, kind="ExternalInput")
    xvR = nc.dram_tensor("xvR", [128, 4 * XVW], f16, kind="ExternalInput")
    wA = nc.dram_tensor("wA", [128, WA], f16, kind="ExternalInput")
    wB = nc.dram_tensor("wB", [128, WB], f16, kind="ExternalInput")
    # partition-major output: out[p, st*768 + c] = result[st*128 + p, c].
    # One DMA per seq-tile PAIR with 3KB lines (the DMA engines are
    # descriptor-bound, so fewer/larger lines beat the row-major layout).
    out = nc.dram_tensor("out", [128, ST * D_MODEL], f16,
                         kind="ExternalOutput")

    with tile.TileContext(nc) as tc:
        with tc.tile_pool(name="const", bufs=1) as cp, \
             tc.tile_pool(name="expp", bufs=4) as ep, \
             tc.tile_pool(name="normp", bufs=2) as np_, \
             tc.tile_pool(name="outp", bufs=3) as op, \
             tc.tile_pool(name="ps_s", bufs=2, space="PSUM") as ps_s, \
             tc.tile_pool(name="ps_c", bufs=2, space="PSUM") as ps_c, \
             tc.tile_pool(name="ps_w", bufs=2, space="PSUM") as ps_w:

            # ---- DMA ----
            wa = cp.tile([128, WA], f16, name="wa")
            nc.sync.dma_start(out=wa[:], in_=wA[:])
            wk_sb = [wa[:, BPK + d * GW:BPK + (d + 1) * GW]
                     for d in range(DC)]
            wq_sb = [wa[:, BPK + DC * GW + d * GW:
                        BPK + DC * GW + (d + 1) * GW] for d in range(DC)]

            xq_sb = cp.tile([128, 2 * XH], f16, name="xq_sb")
            for p3 in range(3):
                nc.sync.dma_start(
                    out=xq_sb[:, p3 * 2048:(p3 + 1) * 2048],
                    in_=xqP[:, p3 * 2048:(p3 + 1) * 2048])
            xk_sb = cp.tile([128, DC * S], f16, name="xk_sb")
            for p3 in range(3):
                nc.sync.dma_start(
                    out=xk_sb[:, p3 * 4096:(p3 + 1) * 4096],
                    in_=xkP[:, p3 * 4096:(p3 + 1) * 4096])

            wb = cp.tile([128, WB], f16, name="wb")
            nc.sync.dma_start(out=wb[:], in_=wB[:])
            wv_sb = [wb[:, d * GW:(d + 1) * GW] for d in range(DC)]
            wo_a = wb[:, DC * GW:DC * GW + D_MODEL]
            wo_bm = wb[:, DC * GW + D_MODEL:WB]

            xvg = [cp.tile([128, XVW], f16, name=f"xv{g}")
                   for g in range(4)]
            for g in range(4):
                nc.sync.dma_start(out=xvg[g][:],
                                  in_=xvR[:, g * XVW:(g + 1) * XVW])
            nc.sync.dma_start(out=xq_sb[:, XH:2 * XH],
                              in_=xqP[:, XH:2 * XH])

            def xk_v(d, c0, w):
                return xk_sb[:, d * 2048 + c0:d * 2048 + c0 + w]

            def xq_v(d, c0, w):
                h, c1 = divmod(c0, 1024)
                return xq_sb[:, h * XH + d * 1024 + c1:
                             h * XH + d * 1024 + c1 + w]

            # biases as f32 scalars (wa holds them as f16)
            bps = cp.tile([128, BPK], f32, name="bps")
            nc.vector.tensor_copy(out=bps[:], in_=wa[:, 0:BPK])
            bq_a, bq_b = bps[:, 0:1], bps[0:64, 1:2]
            bk_a, bk_b = bps[:, 2:3], bps[0:64, 3:4]
            bv_h = [bps[0:64, 4 + h:5 + h] for h in range(HPG)]

            # ---- PE warmup bridging the DMA wait ----
            junk = cp.tile([128, QT], f16, name="junk")
            nc.vector.memset(junk[:], 0.5)
            wupt = [ps_w.tile([128, QT], f32, name="W", tag="w")
                    for _ in range(2)]
            for i in range(24):
                nc.tensor.matmul(wupt[i % 2][:], lhsT=junk[:, 0:128],
                                 rhs=junk[:], start=True, stop=True)
            # dummy exp pulls the ACT exp-table load into the DMA shadow
            escr = cp.tile([128, BPK], f16, name="escr")
            nc.scalar.activation(escr[:], bps[:], Exp, scale=0.001)

            # ---- pre-attention projections, consuming chunk pairs as
            # they land: q passA cols 0:1024, then k passA (all cols) ----
            qT_a = cp.tile([128, S], f16, name="qT_a")
            qT_b = cp.tile([128, S], f16, name="qT_b")
            kT_a = cp.tile([128, S], f16, name="kT_a")
            kT_b = cp.tile([128, S], f16, name="kT_b")

            pjQ = ps_s.tile([128, 2 * QT], f32, name="S", tag="s")
            for d in range(DC):
                for n in range(2):
                    nc.tensor.matmul(
                        pjQ[:, n * QT:(n + 1) * QT],
                        lhsT=wq_sb[d][:, 0:128], rhs=xq_v(d, n * QT, QT),
                        start=(d == 0), stop=(d == DC - 1))
            for n in range(2):
                nc.vector.tensor_scalar_add(
                    qT_a[:, n * QT:(n + 1) * QT],
                    pjQ[:, n * QT:(n + 1) * QT], bq_a)

            pjK = [ps_s.tile([128, 2 * QT], f32, name="S", tag="s")
                   for _ in range(2)]
            for d in range(DC):
                for j2 in range(2):
                    for n in range(2):
                        nc.tensor.matmul(
                            pjK[j2][:, n * QT:(n + 1) * QT],
                            lhsT=wk_sb[d][:, 0:128],
                            rhs=xk_v(d, j2 * 1024 + n * QT, QT),
                            start=(d == 0), stop=(d == DC - 1))
            for j2 in range(2):
                nc.vector.tensor_scalar_add(
                    kT_a[:, j2 * 1024:(j2 + 1) * 1024], pjK[j2][:], bk_a)

            # ---- deferred projection units ----
            fillers = []

            def add_unit(xv_fn, wsel, w_lo, w_hi, b, dst, c0, last, mirror):
                state = {}
                rows = w_hi - w_lo

                def mk(d):
                    def emit():
                        if d == 0:
                            state["pj"] = ps_w.tile([128, QT], f32,
                                                    name="W", tag="w")
                        nc.tensor.matmul(
                            state["pj"][0:rows, :],
                            lhsT=wsel[d][:, w_lo:w_hi],
                            rhs=xv_fn(d, c0, QT),
                            start=(d == 0), stop=(d == DC - 1))
                        if d == DC - 1:
                            nc.vector.tensor_scalar_add(
                                dst[0:rows, c0:c0 + QT],
                                state["pj"][0:rows, :], b)
                            if mirror:
                                nc.sync.dma_start(
                                    out=dst[64:128, c0:c0 + QT],
                                    in_=dst[0:64, c0:c0 + QT])
                    return emit
                for d in range(DC):
                    fillers.append(mk(d))

            for n in range(2):  # q passA cols 1024:2048
                add_unit(xq_v, wq_sb, 0, 128, bq_a, qT_a,
                         1024 + n * QT, False, False)
            for n4 in range(4):  # k passB
                add_unit(xk_v, wk_sb, 128, GW, bk_b, kT_b,
                         n4 * QT, n4 == 3, True)
            for n4 in range(4):  # q passB
                add_unit(xq_v, wq_sb, 128, GW, bq_b, qT_b,
                         n4 * QT, n4 == 3, True)

            # ---- v projection: just-in-time per seq tile inside qt0 ----
            v_sb = [None] * ST

            def v_proj(st):
                g4, s4 = st // 4, st % 4
                pv = ps_w.tile([128, QT], f32, name="W", tag="w")
                for d in range(DC):
                    c0 = (s4 * DC + d) * 128
                    nc.tensor.matmul(pv[:, 0:GW],
                                     lhsT=xvg[g4][:, c0:c0 + 128],
                                     rhs=wv_sb[d][:],
                                     start=(d == 0), stop=(d == DC - 1))
                vt = cp.tile([128, HPG, D_K + 1], f16, name=f"vsb{st}")
                nc.vector.tensor_copy(out=vt[:, :, 0:D_K],
                                      in_=pv[:, 0:GW].rearrange(
                                          "p (h w) -> p h w", h=HPG))
                nc.vector.memset(vt[:, :, D_K:D_K + 1], 1.0)
                v_sb[st] = vt

            # ---- attention state ----
            ctxT_a = [cp.tile([128, QT], f16, name=f"ctxTa{j}")
                      for j in range(NQT)]
            ctxT_b = [cp.tile([128, QT], f16, name=f"ctxTb{j}")
                      for j in range(NQT)]

            ones1 = cp.tile([1, D_K], f16, name="ones1")
            nc.vector.memset(ones1[:], 1.0)

            def normalize(C, h, qt):
                # Copy the accumulator (and denominator row) to SBUF first
                # so the PSUM bank frees early. h1's copy lands at
                # partitions 64:128 so the multiply's SBUF operands share a
                # start partition. The final (tail) normalize broadcasts the
                # reciprocal via a PE ones-matmul instead of GPSIMD — the PE
                # is idle there and it keeps the clock gate open.
                base = 64 if h == 1 else 0
                tail = h == 2 and qt == NQT - 1
                ctx_dst = (ctxT_a[qt][0:64] if h == 0 else
                           ctxT_a[qt][64:128] if h == 1 else
                           ctxT_b[qt][0:64])
                den = np_.tile([1, QT], f32, name="den", tag="den")
                nc.vector.tensor_copy(out=den[:], in_=C[D_K:D_K + 1, :])
                Cc = np_.tile([128, QT], f32, name="Cc", tag="cc")
                nc.vector.tensor_copy(out=Cc[base:base + D_K, :],
                                      in_=C[0:D_K, :])
                r16 = np_.tile([1, QT], f16, name="r16", tag="r16")
                r = np_.tile([1, QT], f32, name="r", tag="r")
                nc.vector.reciprocal_approx_fast(out=r[:], in_=den[:])
                if tail:
                    nc.vector.tensor_copy(out=r16[:], in_=r[:])
                    bcp = ps_c.tile([128, QT], f32, name="C", tag="c")
                    nc.tensor.matmul(bcp[0:D_K, :], lhsT=ones1[:],
                                     rhs=r16[:], start=True, stop=True)
                    bc_ap = bcp[0:D_K, :]
                else:
                    bc = np_.tile([128, QT], f32, name="bc", tag="bc")
                    nc.gpsimd.partition_broadcast(bc[:], r[:])
                    bc_ap = bc[base:base + D_K, :]
                nc.vector.tensor_tensor(out=ctx_dst[:],
                                        in0=Cc[base:base + D_K, :],
                                        in1=bc_ap,
                                        op=mult)
                nc.vector.tensor_scalar_add(ctx_dst[:], ctx_dst[:], bv_h[h])
                if h == 2 and not tail:
                    # mirror so out_proj b-matmuls can alternate row groups
                    nc.sync.dma_start(out=ctxT_b[qt][64:128, :],
                                      in_=ctxT_b[qt][0:64, :])

            osb2 = [None]  # current [128, 1536] tile covering an st pair

            def ou_a(qt, u, po):
                st = u // 2
                ws = slice(st * 128, (st + 1) * 128)
                ns = slice((u % 2) * 384, (u % 2) * 384 + 384)
                nc.tensor.matmul(po[:], lhsT=ctxT_a[qt][:, ws],
                                 rhs=wo_a[:, ns], start=True, stop=False)

            def ou_b(qt, u, po, tail):
                st = u // 2
                ws = slice(st * 128, (st + 1) * 128)
                ns = slice((u % 2) * 384, (u % 2) * 384 + 384)
                rb = (slice(0, 64) if (u % 2 == 0 or tail)
                      else slice(64, 128))
                nc.tensor.matmul(po[:], lhsT=ctxT_b[qt][rb, ws],
                                 rhs=wo_bm[rb, ns], start=False, stop=True)
                oc = (u % 4) * 384
                if tail and u % 2 == 1:
                    nc.scalar.copy(osb2[0][:, oc:oc + 384], po[:])
                else:
                    nc.vector.tensor_copy(out=osb2[0][:, oc:oc + 384],
                                          in_=po[:])
                if u % 4 == 3:
                    c0 = (qt * 4 + (u // 4) * 2) * D_MODEL
                    nc.sync.dma_start(out=out[:, c0:c0 + 2 * D_MODEL],
                                      in_=osb2[0][:])

            def out_unit(qt, u, po=None, tail=False):
                if u % 4 == 0:
                    osb2[0] = op.tile([128, 2 * D_MODEL], f16, name="osb")
                if po is None:
                    po = ps_w.tile([128, QT], f32, name="W",
                                   tag="w")[:, 0:384]
                ou_a(qt, u, po)
                ou_b(qt, u, po, tail)

            # ---- the flat attention stream ----
            # phase descriptors: ("h01", qt) x4 then ("h2", qt) x4
            steps = []
            for qt in range(NQT):
                steps += [("h01", qt, kc) for kc in range(KC)]
            for qt in range(NQT):
                steps += [("h2", qt, kc2) for kc2 in range(KC // 2)]

            Cs = {}   # (kind, qt) -> accumulator tile(s)
            S2q = [None] * len(steps)
            tail_po = []

            def emit_scores(i):
                kind, qt, kc = steps[i]
                S2 = ps_s.tile([128, 2 * QT], f32, name="S", tag="s")
                qs = slice(qt * QT, (qt + 1) * QT)
                if kind == "h01":
                    ks = slice(kc * 128, (kc + 1) * 128)
                    nc.tensor.matmul(S2[:, 0:QT], lhsT=kT_a[0:64, ks],
                                     rhs=qT_a[0:64, qs])
                    nc.tensor.matmul(S2[:, QT:2 * QT],
                                     lhsT=kT_a[64:128, ks],
                                     rhs=qT_a[64:128, qs])
                else:
                    for ii in (0, 1):
                        kcc = 2 * kc + ii
                        rg = slice(64 * ii, 64 * ii + 64)
                        nc.tensor.matmul(
                            S2[:, ii * QT:(ii + 1) * QT],
                            lhsT=kT_b[rg, kcc * 128:(kcc + 1) * 128],
                            rhs=qT_b[rg, qs])
                S2q[i] = S2

            emit_scores(0)
            for i, (kind, qt, kc) in enumerate(steps):
                e2 = ep.tile([128, 2 * QT], f16, name="expT")
                nc.scalar.activation(e2[:], S2q[i][:], Exp, scale=0.125)
                S2q[i] = None
                if i + 1 < len(steps):
                    emit_scores(i + 1)
                # PE-slack extras
                if kind == "h01":
                    if qt == 0:
                        v_proj(kc)
                    else:
                        rate = 2 if (qt == 1 and kc < 4) else 1
                        for _ in range(rate):
                            if fillers:
                                fillers.pop(0)()
                else:
                    if qt > 0:
                        out_unit(qt - 1, kc)
                    elif fillers:
                        fillers.pop(0)()
                # ctx
                if kind == "h01":
                    if kc == 0:
                        Cs[qt] = {h: ps_c.tile([128, QT], f32, name="C",
                                               tag="c") for h in (0, 1)}
                    for h in (0, 1):
                        nc.tensor.matmul(Cs[qt][h][0:D_K + 1, :],
                                         lhsT=v_sb[kc][:, h, :],
                                         rhs=e2[:, h * QT:(h + 1) * QT],
                                         start=(kc == 0), stop=(kc == KC - 1))
                    if kc == KC - 1:
                        for h in (0, 1):
                            normalize(Cs[qt][h], h, qt)
                else:
                    if kc == 0:
                        Cs[("h2", qt)] = ps_c.tile([128, QT], f32,
                                                   name="C", tag="c")
                    C2 = Cs[("h2", qt)]
                    for ii in (0, 1):
                        kcc = 2 * kc + ii
                        nc.tensor.matmul(C2[0:D_K + 1, :],
                                         lhsT=v_sb[kcc][:, 2, :],
                                         rhs=e2[:, ii * QT:(ii + 1) * QT],
                                         start=(kcc == 0),
                                         stop=(kcc == KC - 1))
                    if kc == KC // 2 - 1:
                        if qt == NQT - 1:
                            # prestage the tail out_proj's ctxT_a halves:
                            # they only need ctxT_a, so they run during the
                            # normalize chain, keeping the PE warm. Borrow
                            # the idle score banks + w + c pools so seven
                            # accumulators are in flight.
                            tts = [ps_s.tile([128, 2 * QT], f32, name="S",
                                             tag="s") for _ in range(2)]
                            tail_po.extend([
                                tts[0][:, 0:384], tts[0][:, QT:QT + 384],
                                tts[1][:, 0:384], tts[1][:, QT:QT + 384]])
                            for _ in range(2):
                                tail_po.append(ps_w.tile(
                                    [128, QT], f32, name="W",
                                    tag="w")[:, 0:384])
                            tail_po.append(ps_c.tile(
                                [128, QT], f32, name="C", tag="c")[:, 0:384])
                            for u in range(7):
                                ou_a(NQT - 1, u, tail_po[u])
                        normalize(C2, 2, qt)

            # ---- tail: qt3's out_proj b-halves + copies + DMAs ----
            for u in range(4):
                if u == 0:
                    osb2[0] = op.tile([128, 2 * D_MODEL], f16, name="osb")
                ou_b(NQT - 1, u, tail_po[u], True)
            osb2[0] = op.tile([128, 2 * D_MODEL], f16, name="osb")
            ou_b(NQT - 1, 4, tail_po[4], True)
            ou_b(NQT - 1, 5, tail_po[5], True)
            p7 = ps_w.tile([128, QT], f32, name="W", tag="w")[:, 0:384]
            ou_a(NQT - 1, 7, p7)
            ou_b(NQT - 1, 6, tail_po[6], True)
            ou_b(NQT - 1, 7, p7, True)

    nc.compile()
    return nc


def _get_program():
    global _PROGRAM
    if _PROGRAM is None:
        _PROGRAM = _build_program()
    return _PROGRAM


def make_in_maps(query, key, value, Wq, bq, Wk, bk, Wv, bv, Wo, bo):
    """Build the 8 per-core input maps (host-side shard + pack + cast)."""
    q32 = np.asarray(query, np.float32)
    k32 = np.asarray(key, np.float32)
    v32 = np.asarray(value, np.float32)

    def pack_q(xT):
        # [768, 2048] -> [128, 2*6144]: halves x chunks x 1024
        return np.ascontiguousarray(
            xT.reshape(DC, 128, 2, 1024).transpose(2, 1, 0, 3)
        ).reshape(2, 128, XH).transpose(1, 0, 2).reshape(128, 2 * XH)

    def pack_k(xT):
        # [768, 2048] -> [128, 6*2048]: d-major full-width chunks
        return np.ascontiguousarray(
            xT.reshape(DC, 128, S).transpose(1, 0, 2)).reshape(128, DC * S)

    def pack_v(xT):
        # [768, 2048] -> [128, 16*768]: seq-tile-major
        return np.ascontiguousarray(
            xT.reshape(DC, 128, ST, 128).transpose(1, 2, 0, 3)
        ).reshape(128, ST * DC * 128)

    xP = {}
    for b in range(B):
        xP[b] = (pack_q(q32[b].T.astype(np.float16)),
                 pack_k(k32[b].T.astype(np.float16)),
                 pack_v(v32[b].T.astype(np.float16)))
    Wq = np.asarray(Wq, np.float32)
    Wk = np.asarray(Wk, np.float32)
    Wv = np.asarray(Wv, np.float32)
    Wo = np.asarray(Wo, np.float32)
    bq = np.asarray(bq, np.float32)
    bk = np.asarray(bk, np.float32)
    bv = np.asarray(bv, np.float32)
    in_maps = []
    for c in range(N_CORES):
        b, g = divmod(c, G)
        fs = slice(g * GW, (g + 1) * GW)
        xq, xk, xv = xP[b]
        wa = np.zeros((128, WA), np.float16)
        wa[:, 0] = bq[fs][0:128]
        wa[0:64, 1] = bq[fs][128:GW]
        wa[:, 2] = bk[fs][0:128]
        wa[0:64, 3] = bk[fs][128:GW]
        for h in range(HPG):
            wa[0:64, 4 + h] = bv[fs][h * 64:(h + 1) * 64]
        for i, W in enumerate((Wk, Wq)):
            Ws = W[:, fs]
            for d in range(DC):
                c0 = BPK + (i * DC + d) * GW
                wa[:, c0:c0 + GW] = Ws[d * 128:(d + 1) * 128, :].astype(
                    np.float16)
        wbp = np.zeros((128, WB), np.float16)
        Ws = Wv[:, fs]
        for d in range(DC):
            wbp[:, d * GW:(d + 1) * GW] = \
                Ws[d * 128:(d + 1) * 128, :].astype(np.float16)
        Wos = Wo[fs, :]
        wbp[:, DC * GW:DC * GW + D_MODEL] = Wos[0:128, :].astype(np.float16)
        wob = Wos[128:GW, :].astype(np.float16)
        wbp[0:64, DC * GW + D_MODEL:WB] = wob
        wbp[64:128, DC * GW + D_MODEL:WB] = wob
        in_maps.append({
            "xqP": xq, "xkP": xk, "xvR": xv,
            "wA": wa, "wB": wbp,
        })
    return in_maps


def unpack_out(o2):
    """[128, 16*768] partition-major partial -> [2048, 768]."""
    return np.asarray(o2, np.float32).reshape(
        128, ST, D_MODEL).transpose(1, 0, 2).reshape(S, D_MODEL)


def combine_outputs(results, bo):
    """Sum the per-core partial outputs into the full [B, S, D] output."""
    bo = np.asarray(bo, np.float32)
    out = np.zeros((B, S, D_MODEL), np.float32)
    for c in range(N_CORES):
        b = c // G
        out[b] += unpack_out(results[c]["out"])
    out += bo[None, None, :]
    return out


def kernel(**inputs):
    from concourse.bass_utils import run_bass_kernel_spmd

    nc = _get_program()
    in_maps = make_in_maps(**inputs)
    res = run_bass_kernel_spmd(nc, in_maps, list(range(N_CORES)))
    return combine_outputs(res.results, inputs["bo"])


# revision 28
# speedup vs baseline: 1.1359x; 1.0037x over previous
"""Multi-head attention (B=2, S=2048, D=768, H=12) on 8 trn2 NeuronCores.

Sharding: batch x head-group data/tensor parallel. Core c = b*4+g handles
batch b and heads [3g, 3g+3) (a 192-wide slice of the QKV projections and
the matching 192-row slice of Wo). Each core emits a partial [2048, 768]
fp16 output; the host sums the 4 head-group partials per batch and adds bo.

Device schedule. The kernel is dual-roofline (~100us PE streaming, ~97us
ACT exp). The DMA engines sustain ~330GB/s only with large contiguous
lines (they are descriptor-bound at ~10ns/partition-line), so inputs are
host-packed:
  wA  [128, 2312]   biases(f16) | wk | wq            (4.6KB lines)
  xqP [128, 12288]  column halves x chunks x 1024    (4KB-line pair xfers)
  xkP [128, 12288]  d-major chunks x 2048            (8KB-line pair xfers)
  wB  [128, 2688]   wv | wo_a | wo_b mirrored        (5.4KB lines)
  xvR [128, 12288]  seq-tile-quad groups             (6KB lines)
DMA order: wA, xq-half0 (3 chunk-pair transfers), xk (3 pair transfers),
wB, xv groups, xq-half1. Projections consume each pair as it lands; the
first exp fires ~23us in. Warmup junk matmuls bridge the DMA wait so the
PE HAM clock gate stays open.

Only k passA and q passA columns 0:1024 run before attention. The rest
(q passA cols 1024:2048, k/q passB) is deferred into attention PE slack
as six-matmul units cycling through the 2-bank "w" PSUM pool; the v
projection runs just-in-time per seq tile inside qt0.

The attention itself is ONE flat software-pipelined stream over phases
h01(qt0..3) then h2(qt0..3): at every step the NEXT step's score matmuls
are emitted before this step's ctx matmuls — across phase boundaries too
— so the scalar engine's monotonic semaphore wait for exp[i+1] never
covers ctx[i] and the exp stream never drains at a boundary. h2 phases
carry the previous q-tile's output projection, one (st, ns) unit per
iteration; out_proj(qt3) is the tail (borrowing idle score banks so four
units are in flight, copies alternating Scalar/Vector). normalize()
copies the accumulator to SBUF immediately so the PSUM bank frees early;
ctxT_b and wo_b are mirrored into partitions 64:127 so out_proj b-matmuls
alternate PE row groups.
"""

import numpy as np

D_MODEL = 768
NUM_HEADS = 12
D_K = 64
B = 2
S = 2048
N_CORES = 8
G = 4              # head groups (cores per batch)
GW = D_MODEL // G  # 192 features per group = 3 heads
HPG = 3            # heads per group
DC = D_MODEL // 128  # 6 d_model chunks
QT = 512           # q-tile width
NQT = S // QT      # 4
KC = S // 128      # 16 k chunks
ST = S // 128      # 16 seq tiles
BPK = 8            # packed bias columns
WA = BPK + 2 * DC * GW        # 2312: bias | wk | wq
WB = DC * GW + 2 * D_MODEL    # 2688: wv | wo_a | wo_b(mirrored)
XH = DC * 1024                # 6144: one xq half (6 chunks x 1024 seq)
XVW = 4 * DC * 128            # 3072: one xv group (4 seq tiles)

_PROGRAM = None


def _build_program():
    from concourse import bacc, tile
    import concourse.mybir as mybir

    f16 = mybir.dt.float16
    f32 = mybir.dt.float32
    Exp = mybir.ActivationFunctionType.Exp
    mult = mybir.AluOpType.mult

    nc = bacc.Bacc("TRN2", target_bir_lowering=False, debug=False,
                   enable_asserts=False)

    xkP = nc.dram_tensor("xkP", [128, DC * S], f16, kind="ExternalInput")
    xqP = nc.dram_tensor("xqP", [128, 2 * XH], f16, kind="ExternalInput")
    xvR = nc.dram_tensor("xvR", [128, 4 * XVW], f16, kind="ExternalInput")
    wA = nc.dram_tensor("wA", [128, WA], f16, kind="ExternalInput")
    wB = nc.dram_tensor("wB", [128, WB], f16, kind="ExternalInput")
    # partition-major output: out[p, st*768 + c] = result[st*128 + p, c].
    # One DMA per seq-tile PAIR with 3KB lines (the DMA engines are
    # descriptor-bound, so fewer/larger lines beat the row-major layout).
    out = nc.dram_tensor("out", [128, ST * D_MODEL], f16,
                         kind="ExternalOutput")

    with tile.TileContext(nc) as tc:
        with tc.tile_pool(name="const", bufs=1) as cp, \
             tc.tile_pool(name="expp", bufs=4) as ep, \
             tc.tile_pool(name="normp", bufs=2) as np_, \
             tc.tile_pool(name="outp", bufs=3) as op, \
             tc.tile_pool(name="ps_s", bufs=2, space="PSUM") as ps_s, \
             tc.tile_pool(name="ps_c", bufs=2, space="PSUM") as ps_c, \
             tc.tile_pool(name="ps_w", bufs=2, space="PSUM") as ps_w:

            # ---- DMA ----
            wa = cp.tile([128, WA], f16, name="wa")
            nc.sync.dma_start(out=wa[:], in_=wA[:])
            wk_sb = [wa[:, BPK + d * GW:BPK + (d + 1) * GW]
                     for d in range(DC)]
            wq_sb = [wa[:, BPK + DC * GW + d * GW:
                        BPK + DC * GW + (d + 1) * GW] for d in range(DC)]

            xq_sb = cp.tile([128, 2 * XH], f16, name="xq_sb")
            for p3 in range(3):
                nc.sync.dma_start(
                    out=xq_sb[:, p3 * 2048:(p3 + 1) * 2048],
                    in_=xqP[:, p3 * 2048:(p3 + 1) * 2048])
            xk_sb = cp.tile([128, DC * S], f16, name="xk_sb")
            for p3 in range(3):
                nc.sync.dma_start(
                    out=xk_sb[:, p3 * 4096:(p3 + 1) * 4096],
                    in_=xkP[:, p3 * 4096:(p3 + 1) * 4096])

            wb = cp.tile([128, WB], f16, name="wb")
            nc.sync.dma_start(out=wb[:], in_=wB[:])
            wv_sb = [wb[:, d * GW:(d + 1) * GW] for d in range(DC)]
            wo_a = wb[:, DC * GW:DC * GW + D_MODEL]
            wo_bm = wb[:, DC * GW + D_MODEL:WB]

            xvg = [cp.tile([128, XVW], f16, name=f"xv{g}")
                   for g in range(4)]
            for g in range(4):
                nc.sync.dma_start(out=xvg[g][:],
                                  in_=xvR[:, g * XVW:(g + 1) * XVW])
            nc.sync.dma_start(out=xq_sb[:, XH:2 * XH],
                              in_=xqP[:, XH:2 * XH])

            def xk_v(d, c0, w):
                return xk_sb[:, d * 2048 + c0:d * 2048 + c0 + w]

            def xq_v(d, c0, w):
                h, c1 = divmod(c0, 1024)
                return xq_sb[:, h * XH + d * 1024 + c1:
                             h * XH + d * 1024 + c1 + w]

            # biases as f32 scalars (wa holds them as f16)
            bps = cp.tile([128, BPK], f32, name="bps")
            nc.vector.tensor_copy(out=bps[:], in_=wa[:, 0:BPK])
            bq_a, bq_b = bps[:, 0:1], bps[0:64, 1:2]
            bk_a, bk_b = bps[:, 2:3], bps[0:64, 3:4]
            bv_h = [bps[0:64, 4 + h:5 + h] for h in range(HPG)]

            # ---- PE warmup bridging the DMA wait ----
            junk = cp.tile([128, QT], f16, name="junk")
            nc.vector.memset(junk[:], 0.5)
            wupt = [ps_w.tile([128, QT], f32, name="W", tag="w")
                    for _ in range(2)]
            for i in range(24):
                nc.tensor.matmul(wupt[i % 2][:], lhsT=junk[:, 0:128],
                                 rhs=junk[:], start=True, stop=True)
            # dummy exp pulls the ACT exp-table load into the DMA shadow
            escr = cp.tile([128, BPK], f16, name="escr")
            nc.scalar.activation(escr[:], bps[:], Exp, scale=0.001)

            # ---- pre-attention projections, consuming chunk pairs as
            # they land: q passA cols 0:1024, then k passA (all cols) ----
            qT_a = cp.tile([128, S], f16, name="qT_a")
            qT_b = cp.tile([128, S], f16, name="qT_b")
            kT_a = cp.tile([128, S], f16, name="kT_a")
            kT_b = cp.tile([128, S], f16, name="kT_b")

            pjQ = ps_s.tile([128, 2 * QT], f32, name="S", tag="s")
            for d in range(DC):
                for n in range(2):
                    nc.tensor.matmul(
                        pjQ[:, n * QT:(n + 1) * QT],
                        lhsT=wq_sb[d][:, 0:128], rhs=xq_v(d, n * QT, QT),
                        start=(d == 0), stop=(d == DC - 1))
            for n in range(2):
                nc.vector.tensor_scalar_add(
                    qT_a[:, n * QT:(n + 1) * QT],
                    pjQ[:, n * QT:(n + 1) * QT], bq_a)

            pjK = [ps_s.tile([128, 2 * QT], f32, name="S", tag="s")
                   for _ in range(2)]
            for d in range(DC):
                for j2 in range(2):
                    for n in range(2):
                        nc.tensor.matmul(
                            pjK[j2][:, n * QT:(n + 1) * QT],
                            lhsT=wk_sb[d][:, 0:128],
                            rhs=xk_v(d, j2 * 1024 + n * QT, QT),
                            start=(d == 0), stop=(d == DC - 1))
            for j2 in range(2):
                for n in range(2):
                    cs = slice(j2 * 1024 + n * QT, j2 * 1024 + (n + 1) * QT)
                    nc.vector.tensor_scalar_add(
                        kT_a[:, cs], pjK[j2][:, n * QT:(n + 1) * QT], bk_a)

            # ---- deferred projection units ----
            fillers = []

            def add_unit(xv_fn, wsel, w_lo, w_hi, b, dst, c0, last, mirror):
                state = {}
                rows = w_hi - w_lo

                def mk(d):
                    def emit():
                        if d == 0:
                            state["pj"] = ps_w.tile([128, QT], f32,
                                                    name="W", tag="w")
                        nc.tensor.matmul(
                            state["pj"][0:rows, :],
                            lhsT=wsel[d][:, w_lo:w_hi],
                            rhs=xv_fn(d, c0, QT),
                            start=(d == 0), stop=(d == DC - 1))
                        if d == DC - 1:
                            nc.vector.tensor_scalar_add(
                                dst[0:rows, c0:c0 + QT],
                                state["pj"][0:rows, :], b)
                            if mirror:
                                nc.sync.dma_start(
                                    out=dst[64:128, c0:c0 + QT],
                                    in_=dst[0:64, c0:c0 + QT])
                    return emit
                for d in range(DC):
                    fillers.append(mk(d))

            for n in range(2):  # q passA cols 1024:2048
                add_unit(xq_v, wq_sb, 0, 128, bq_a, qT_a,
                         1024 + n * QT, False, False)
            for n4 in range(4):  # k passB
                add_unit(xk_v, wk_sb, 128, GW, bk_b, kT_b,
                         n4 * QT, n4 == 3, True)
            for n4 in range(4):  # q passB
                add_unit(xq_v, wq_sb, 128, GW, bq_b, qT_b,
                         n4 * QT, n4 == 3, True)

            # ---- v projection: just-in-time per seq tile inside qt0 ----
            v_sb = [None] * ST

            def v_proj(st):
                g4, s4 = st // 4, st % 4
                pv = ps_w.tile([128, QT], f32, name="W", tag="w")
                for d in range(DC):
                    c0 = (s4 * DC + d) * 128
                    nc.tensor.matmul(pv[:, 0:GW],
                                     lhsT=xvg[g4][:, c0:c0 + 128],
                                     rhs=wv_sb[d][:],
                                     start=(d == 0), stop=(d == DC - 1))
                vt = cp.tile([128, HPG, D_K + 1], f16, name=f"vsb{st}")
                nc.vector.tensor_copy(out=vt[:, :, 0:D_K],
                                      in_=pv[:, 0:GW].rearrange(
                                          "p (h w) -> p h w", h=HPG))
                nc.vector.memset(vt[:, :, D_K:D_K + 1], 1.0)
                v_sb[st] = vt

            # ---- attention state ----
            ctxT_a = [cp.tile([128, QT], f16, name=f"ctxTa{j}")
                      for j in range(NQT)]
            ctxT_b = [cp.tile([128, QT], f16, name=f"ctxTb{j}")
                      for j in range(NQT)]

            ones1 = cp.tile([1, D_K], f16, name="ones1")
            nc.vector.memset(ones1[:], 1.0)

            def normalize(C, h, qt):
                # Copy the accumulator (and denominator row) to SBUF first
                # so the PSUM bank frees early. h1's copy lands at
                # partitions 64:128 so the multiply's SBUF operands share a
                # start partition. The final (tail) normalize broadcasts the
                # reciprocal via a PE ones-matmul instead of GPSIMD — the PE
                # is idle there and it keeps the clock gate open.
                base = 64 if h == 1 else 0
                tail = h == 2 and qt == NQT - 1
                ctx_dst = (ctxT_a[qt][0:64] if h == 0 else
                           ctxT_a[qt][64:128] if h == 1 else
                           ctxT_b[qt][0:64])
                den = np_.tile([1, QT], f32, name="den", tag="den")
                nc.vector.tensor_copy(out=den[:], in_=C[D_K:D_K + 1, :])
                Cc = np_.tile([128, QT], f32, name="Cc", tag="cc")
                nc.vector.tensor_copy(out=Cc[base:base + D_K, :],
                                      in_=C[0:D_K, :])
                r16 = np_.tile([1, QT], f16, name="r16", tag="r16")
                r = np_.tile([1, QT], f32, name="r", tag="r")
                nc.vector.reciprocal_approx_fast(out=r[:], in_=den[:])
                if tail:
                    nc.vector.tensor_copy(out=r16[:], in_=r[:])
                    bcp = ps_c.tile([128, QT], f32, name="C", tag="c")
                    nc.tensor.matmul(bcp[0:D_K, :], lhsT=ones1[:],
                                     rhs=r16[:], start=True, stop=True)
                    bc_ap = bcp[0:D_K, :]
                else:
                    bc = np_.tile([128, QT], f32, name="bc", tag="bc")
                    nc.gpsimd.partition_broadcast(bc[:], r[:])
                    bc_ap = bc[base:base + D_K, :]
                nc.vector.tensor_tensor(out=ctx_dst[:],
                                        in0=Cc[base:base + D_K, :],
                                        in1=bc_ap,
                                        op=mult)
                nc.vector.tensor_scalar_add(ctx_dst[:], ctx_dst[:], bv_h[h])
                if h == 2 and not tail:
                    # mirror so out_proj b-matmuls can alternate row groups
                    nc.sync.dma_start(out=ctxT_b[qt][64:128, :],
                                      in_=ctxT_b[qt][0:64, :])

            osb2 = [None]  # current [128, 1536] tile covering an st pair

            def ou_a(qt, u, po):
                st = u // 2
                ws = slice(st * 128, (st + 1) * 128)
                ns = slice((u % 2) * 384, (u % 2) * 384 + 384)
                nc.tensor.matmul(po[:], lhsT=ctxT_a[qt][:, ws],
                                 rhs=wo_a[:, ns], start=True, stop=False)

            def ou_b(qt, u, po, tail, sc=False):
                st = u // 2
                ws = slice(st * 128, (st + 1) * 128)
                ns = slice((u % 2) * 384, (u % 2) * 384 + 384)
                rb = (slice(0, 64) if (u % 2 == 0 or tail)
                      else slice(64, 128))
                nc.tensor.matmul(po[:], lhsT=ctxT_b[qt][rb, ws],
                                 rhs=wo_bm[rb, ns], start=False, stop=True)
                oc = (u % 4) * 384
                if sc or (tail and u % 2 == 1):
                    nc.scalar.copy(osb2[0][:, oc:oc + 384], po[:])
                else:
                    nc.vector.tensor_copy(out=osb2[0][:, oc:oc + 384],
                                          in_=po[:])
                if u % 4 == 3:
                    c0 = (qt * 4 + (u // 4) * 2) * D_MODEL
                    nc.sync.dma_start(out=out[:, c0:c0 + 2 * D_MODEL],
                                      in_=osb2[0][:])

            def out_unit(qt, u, po=None, tail=False, sc=False):
                if u % 4 == 0:
                    osb2[0] = op.tile([128, 2 * D_MODEL], f16, name="osb")
                if po is None:
                    po = ps_w.tile([128, QT], f32, name="W",
                                   tag="w")[:, 0:384]
                ou_a(qt, u, po)
                ou_b(qt, u, po, tail, sc)

            # ---- the flat attention stream ----
            # phase descriptors: ("h01", qt) x4 then ("h2", qt) x4
            steps = []
            for qt in range(NQT):
                steps += [("h01", qt, kc) for kc in range(KC)]
            for qt in range(NQT):
                steps += [("h2", qt, kc2) for kc2 in range(KC // 2)]

            Cs = {}   # (kind, qt) -> accumulator tile(s)
            S2q = [None] * len(steps)
            tail_po = []

            def emit_scores(i):
                kind, qt, kc = steps[i]
                S2 = ps_s.tile([128, 2 * QT], f32, name="S", tag="s")
                qs = slice(qt * QT, (qt + 1) * QT)
                if kind == "h01":
                    ks = slice(kc * 128, (kc + 1) * 128)
                    nc.tensor.matmul(S2[:, 0:QT], lhsT=kT_a[0:64, ks],
                                     rhs=qT_a[0:64, qs])
                    nc.tensor.matmul(S2[:, QT:2 * QT],
                                     lhsT=kT_a[64:128, ks],
                                     rhs=qT_a[64:128, qs])
                else:
                    for ii in (0, 1):
                        kcc = 2 * kc + ii
                        rg = slice(64 * ii, 64 * ii + 64)
                        nc.tensor.matmul(
                            S2[:, ii * QT:(ii + 1) * QT],
                            lhsT=kT_b[rg, kcc * 128:(kcc + 1) * 128],
                            rhs=qT_b[rg, qs])
                S2q[i] = S2

            emit_scores(0)
            for i, (kind, qt, kc) in enumerate(steps):
                e2 = ep.tile([128, 2 * QT], f16, name="expT")
                nc.scalar.activation(e2[:], S2q[i][:], Exp, scale=0.125)
                S2q[i] = None
                if i + 1 < len(steps):
                    emit_scores(i + 1)
                # PE-slack extras
                if kind == "h01":
                    if qt == 0:
                        v_proj(kc)
                    else:
                        rate = 2 if (qt == 1 and kc < 4) else 1
                        for _ in range(rate):
                            if fillers:
                                fillers.pop(0)()
                else:
                    if qt > 0:
                        # the last two injected units copy via the (by then
                        # idle) scalar engine so the vector queue is clear
                        # for the tail normalize
                        out_unit(qt - 1, kc,
                                 sc=(qt == NQT - 1 and kc >= 6))
                    elif fillers:
                        fillers.pop(0)()
                # ctx
                if kind == "h01":
                    if kc == 0:
                        Cs[qt] = {h: ps_c.tile([128, QT], f32, name="C",
                                               tag="c") for h in (0, 1)}
                    for h in (0, 1):
                        nc.tensor.matmul(Cs[qt][h][0:D_K + 1, :],
                                         lhsT=v_sb[kc][:, h, :],
                                         rhs=e2[:, h * QT:(h + 1) * QT],
                                         start=(kc == 0), stop=(kc == KC - 1))
                    if kc == KC - 1:
                        for h in (0, 1):
                            normalize(Cs[qt][h], h, qt)
                else:
                    if kc == 0:
                        Cs[("h2", qt)] = ps_c.tile([128, QT], f32,
                                                   name="C", tag="c")
                    C2 = Cs[("h2", qt)]
                    for ii in (0, 1):
                        kcc = 2 * kc + ii
                        nc.tensor.matmul(C2[0:D_K + 1, :],
                                         lhsT=v_sb[kcc][:, 2, :],
                                         rhs=e2[:, ii * QT:(ii + 1) * QT],
                                         start=(kcc == 0),
                                         stop=(kcc == KC - 1))
                    if kc == KC // 2 - 1:
                        if qt == NQT - 1:
                            # prestage the tail out_proj's ctxT_a halves:
                            # they only need ctxT_a, so they run during the
                            # normalize chain, keeping the PE warm. Borrow
                            # the idle score banks + w + c pools so seven
                            # accumulators are in flight.
                            tts = [ps_s.tile([128, 2 * QT], f32, name="S",
                                             tag="s") for _ in range(2)]
                            tail_po.extend([
                                tts[0][:, 0:384], tts[0][:, QT:QT + 384],
                                tts[1][:, 0:384], tts[1][:, QT:QT + 384]])
                            for _ in range(2):
                                tail_po.append(ps_w.tile(
                                    [128, QT], f32, name="W",
                                    tag="w")[:, 0:384])
                            tail_po.append(ps_c.tile(
                                [128, QT], f32, name="C", tag="c")[:, 0:384])
                            for u in range(7):
                                ou_a(NQT - 1, u, tail_po[u])
                        normalize(C2, 2, qt)

            # ---- tail: qt3's out_proj b-halves + copies + DMAs ----
            for u in range(4):
                if u == 0:
                    osb2[0] = op.tile([128, 2 * D_MODEL], f16, name="osb")
                ou_b(NQT - 1, u, tail_po[u], True)
            osb2[0] = op.tile([128, 2 * D_MODEL], f16, name="osb")
            ou_b(NQT - 1, 4, tail_po[4], True)
            ou_b(NQT - 1, 5, tail_po[5], True)
            p7 = ps_w.tile([128, QT], f32, name="W", tag="w")[:, 0:384]
            ou_a(NQT - 1, 7, p7)
            ou_b(NQT - 1, 6, tail_po[6], True)
            ou_b(NQT - 1, 7, p7, True)

    nc.compile()
    return nc


def _get_program():
    global _PROGRAM
    if _PROGRAM is None:
        _PROGRAM = _build_program()
    return _PROGRAM


def make_in_maps(query, key, value, Wq, bq, Wk, bk, Wv, bv, Wo, bo):
    """Build the 8 per-core input maps (host-side shard + pack + cast)."""
    q32 = np.asarray(query, np.float32)
    k32 = np.asarray(key, np.float32)
    v32 = np.asarray(value, np.float32)

    def pack_q(xT):
        # [768, 2048] -> [128, 2*6144]: halves x chunks x 1024
        return np.ascontiguousarray(
            xT.reshape(DC, 128, 2, 1024).transpose(2, 1, 0, 3)
        ).reshape(2, 128, XH).transpose(1, 0, 2).reshape(128, 2 * XH)

    def pack_k(xT):
        # [768, 2048] -> [128, 6*2048]: d-major full-width chunks
        return np.ascontiguousarray(
            xT.reshape(DC, 128, S).transpose(1, 0, 2)).reshape(128, DC * S)

    def pack_v(xT):
        # [768, 2048] -> [128, 16*768]: seq-tile-major
        return np.ascontiguousarray(
            xT.reshape(DC, 128, ST, 128).transpose(1, 2, 0, 3)
        ).reshape(128, ST * DC * 128)

    xP = {}
    for b in range(B):
        xP[b] = (pack_q(q32[b].T.astype(np.float16)),
                 pack_k(k32[b].T.astype(np.float16)),
                 pack_v(v32[b].T.astype(np.float16)))
    Wq = np.asarray(Wq, np.float32)
    Wk = np.asarray(Wk, np.float32)
    Wv = np.asarray(Wv, np.float32)
    Wo = np.asarray(Wo, np.float32)
    bq = np.asarray(bq, np.float32)
    bk = np.asarray(bk, np.float32)
    bv = np.asarray(bv, np.float32)
    in_maps = []
    for c in range(N_CORES):
        b, g = divmod(c, G)
        fs = slice(g * GW, (g + 1) * GW)
        xq, xk, xv = xP[b]
        wa = np.zeros((128, WA), np.float16)
        wa[:, 0] = bq[fs][0:128]
        wa[0:64, 1] = bq[fs][128:GW]
        wa[:, 2] = bk[fs][0:128]
        wa[0:64, 3] = bk[fs][128:GW]
        for h in range(HPG):
            wa[0:64, 4 + h] = bv[fs][h * 64:(h + 1) * 64]
        for i, W in enumerate((Wk, Wq)):
            Ws = W[:, fs]
            for d in range(DC):
                c0 = BPK + (i * DC + d) * GW
                wa[:, c0:c0 + GW] = Ws[d * 128:(d + 1) * 128, :].astype(
                    np.float16)
        wbp = np.zeros((128, WB), np.float16)
        Ws = Wv[:, fs]
        for d in range(DC):
            wbp[:, d * GW:(d + 1) * GW] = \
                Ws[d * 128:(d + 1) * 128, :].astype(np.float16)
        Wos = Wo[fs, :]
        wbp[:, DC * GW:DC * GW + D_MODEL] = Wos[0:128, :].astype(np.float16)
        wob = Wos[128:GW, :].astype(np.float16)
        wbp[0:64, DC * GW + D_MODEL:WB] = wob
        wbp[64:128, DC * GW + D_MODEL:WB] = wob
        in_maps.append({
            "xqP": xq, "xkP": xk, "xvR": xv,
            "wA": wa, "wB": wbp,
        })
    return in_maps


def unpack_out(o2):
    """[128, 16*768] partition-major partial -> [2048, 768]."""
    return np.asarray(o2, np.float32).reshape(
        128, ST, D_MODEL).transpose(1, 0, 2).reshape(S, D_MODEL)


def combine_outputs(results, bo):
    """Sum the per-core partial outputs into the full [B, S, D] output."""
    bo = np.asarray(bo, np.float32)
    out = np.zeros((B, S, D_MODEL), np.float32)
    for c in range(N_CORES):
        b = c // G
        out[b] += unpack_out(results[c]["out"])
    out += bo[None, None, :]
    return out


def kernel(**inputs):
    from concourse.bass_utils import run_bass_kernel_spmd

    nc = _get_program()
    in_maps = make_in_maps(**inputs)
    res = run_bass_kernel_spmd(nc, in_maps, list(range(N_CORES)))
    return combine_outputs(res.results, inputs["bo"])
